# revision 63
# baseline (speedup 1.0000x reference)
"""Trainium2 Bass kernel for nn_BertSelfAttention_79577154060613.

Block-sparse BERT self-attention (block-diagonal over 10 candidate blocks of
64 tokens + dense global columns for 128 term tokens), data-parallel over
batch across 8 NeuronCores (2 batches per core).

Key algorithmic trick: the reference multiplies scores by the mask (masked
entries become exactly 0, not -inf), so softmax gives each masked key weight
exp(0)=1. For a query in block c:
    ctx = (sum_{k in block c | terms} e^{s_k} v_k + sum_{c' != c} Vsum_c') / Z
    Z   = sum_{k in block c | terms} e^{s_k} + 9*64
where Vsum_c' are per-head, per-block sums of candidate value rows. This
turns 768-wide attention into 192-wide attention plus one small K=10 matmul
(lhsT = 1 - one_hot(c)) per query tile.

The three projections run as fp8e4 DoubleRow matmuls (two fp8 elements per
PE cell -> half the instructions at twice the rate): the host pre-scales x
by 16 and W^T by 32 (power-of-two, exact) and packs the H=768 contraction
as 3 [128 partitions, 2, .] chunk-pair 3D APs. (Scores stay bf16: walrus
rejects DoubleRow combined with PE column tiling, which the 64-partition
block-score matmuls need.) Everything downstream carries the combined 512x
scale: the exp scale folds in 2^-21 exactly, V/Vsums are stored 512x with
the softmax-denominator ones column set to 512 (and 64*512 in Vsums), so
the final reciprocal-multiply cancels the scale with zero extra
instructions. fp8 rounding noise averages
out across the 768-key softmax (<1e-3 relative) everywhere except the term
value rows, which pass through to the output verbatim - those are recomputed
in bf16 (a small [128, H] matmul) with the bias folded in as a rank-1 matmul.

PSUM layout: one [128, 1024] 2-bank tag (psW, 3 bufs) hosts every projection
and score group so each drains with ONE wide ACT/DVE op (engine init
latency ~150-185ns dominates small copies; GpSimd has no PSUM port at all).
A head's scores pack term chunks at [0:512],[512:640] and all 10 block
products at [640:960] - one 960-wide exp per head. PV accumulates in psC
(2 single-bank bufs); each bank hosts its groups opened by full-height
matmuls and closed by a rank-1 +0 whose stop ends the group. The emission
order software-pipelines the two batches: scores/PV stages alternate and
projection units of the other batch fill every gap, so exp work (ACT),
PSUM drains (ACT+DVE split) and matmuls (PE) overlap throughout.
"""

import numpy as np
import ml_dtypes

import concourse.bass as bass
import concourse.mybir as mybir
import concourse.tile as tile
from concourse import bacc
from concourse.bass_utils import run_bass_kernel_spmd

# Problem dims (hardcoded per contract)
B, CDD, L, T, H, NH = 16, 10, 64, 128, 768, 12
DH = H // NH  # 64
S = CDD * L + T  # 768
NQ = CDD * L  # 640
P = 128
NCORES = 8
BL = B // NCORES  # 2 batches per core
KT = H // P  # 6 contraction tiles
KP = KT // 2  # 3 DoubleRow chunk pairs
FP32 = mybir.dt.float32
BF16 = mybir.dt.bfloat16
F8 = mybir.dt.float8e4
AF = mybir.ActivationFunctionType
ALU = mybir.AluOpType
DR = mybir.MatmulPerfMode.DoubleRow
HGS = 4  # heads per attention group (= heads per Q/K o-tile)
NHG = NH // HGS  # 3 groups
VW = DH + 1  # value width per head incl. denominator column (65)
SX = 16.0  # fp8 scale on x
SWT = 32.0  # fp8 scale on W
SC = SX * SWT  # combined 512x scale carried through the kernel
ESCALE = 0.125 / (SC * SC)  # exp scale: 1/sqrt(DH) / 512^2 = 2^-21 (exact)
SB = 640  # block-scores base column inside a head's score tile


def _build_program():
    nc = bacc.Bacc(
        "TRN2", target_bir_lowering=False, debug=False, num_devices=NCORES
    )
    x8 = nc.dram_tensor("x8", [BL, P, KT * S], F8, kind="ExternalInput").ap()
    xt16 = nc.dram_tensor("xt16", [BL, P, KT * T], BF16, kind="ExternalInput").ap()
    w8q = nc.dram_tensor("w8q", [P, KT * H], F8, kind="ExternalInput").ap()
    w8k = nc.dram_tensor("w8k", [P, KT * H], F8, kind="ExternalInput").ap()
    w8v = nc.dram_tensor("w8v", [P, KT * H], F8, kind="ExternalInput").ap()
    wv16 = nc.dram_tensor("wv16", [P, KT * H], BF16, kind="ExternalInput").ap()
    bcolqk = nc.dram_tensor("bcolqk", [P, 2 * KT], FP32, kind="ExternalInput").ap()
    bvb512 = nc.dram_tensor("bvb512", [P, H], BF16, kind="ExternalInput").ap()
    out = nc.dram_tensor("out", [BL, S, H], BF16, kind="ExternalOutput").ap()

    with tile.TileContext(nc) as tc:
        _emit(tc, nc, x8, xt16, w8q, w8k, w8v, wv16, bcolqk, bvb512, out)
    nc.compile()
    return nc


def _emit(tc, nc, x8, xt16, w8q, w8k, w8v, wv16, bcolqk, bvb512, out):
    from collections import deque
    from contextlib import ExitStack

    ctx = ExitStack()
    with ctx:
        cpool = ctx.enter_context(tc.tile_pool(name="consts", bufs=1))
        wpool = ctx.enter_context(tc.tile_pool(name="weights", bufs=1))
        xtp = ctx.enter_context(tc.tile_pool(name="xt", bufs=2))
        qkv = ctx.enter_context(tc.tile_pool(name="qkv", bufs=2))
        sep = ctx.enter_context(tc.tile_pool(name="se", bufs=4))
        osp = ctx.enter_context(tc.tile_pool(name="osb", bufs=2))
        smp = ctx.enter_context(tc.tile_pool(name="small", bufs=2))
        psp = ctx.enter_context(tc.tile_pool(name="psum", bufs=1, space="PSUM"))

        # ---- constants ----
        onesrow = cpool.tile([1, P], BF16)  # 1.0 row (group-closer rank-1 lhsT)
        nc.gpsimd.memset(onesrow[:], 1.0)
        # tiny activation at t=0 pulls the implicit ACT table load into the
        # initial DMA wait instead of blocking the first real copy
        actwarm = cpool.tile([1, P], BF16)
        nc.scalar.activation(actwarm[:], onesrow[:], AF.Exp)
        invrow = cpool.tile([1, P], BF16)  # 1/512 row (vterm bias rank-1 lhsT)
        nc.gpsimd.memset(invrow[:], 1.0 / SC)
        zrow = cpool.tile([1, 1], BF16)  # 0.0 (group-closer rank-1 rhs)
        nc.gpsimd.memset(zrow[:], 0.0)
        # notselC[p, c*64+j] = 0 if p == c else 1  (p in 0..9)
        notselC = cpool.tile([CDD, NQ], BF16)
        nc.gpsimd.memset(notselC[:], 1.0)
        nc.gpsimd.affine_select(
            out=notselC.rearrange("p (c j) -> p c j", j=L),
            in_=notselC.rearrange("p (c j) -> p c j", j=L),
            compare_op=ALU.not_equal,
            fill=0.0,
            base=0,
            pattern=[[-1, CDD], [0, L]],
            channel_multiplier=1,
        )
        # block-membership indicator for Vsums: G[p, j] = 1 iff j-10 == p//64
        G = cpool.tile([P, 20], BF16)
        nc.gpsimd.memset(G[:], 0.0)
        nc.gpsimd.memset(G[0:64, 10:11], 1.0)
        nc.gpsimd.memset(G[64:128, 11:12], 1.0)

        # ---- weights, biases, x (all layouts host-prepared) ----
        wq_sb = wpool.tile([P, KT, H], F8, tag="wq", name="wq")
        wk_sb = wpool.tile([P, KT, H], F8, tag="wk", name="wk")
        wv_sb = wpool.tile([P, KT, H], F8, tag="wv", name="wv")
        wv16_sb = wpool.tile([P, KT, H], BF16, tag="wv16", name="wv16")
        bcol = cpool.tile([P, 2 * KT], FP32)
        bvb = cpool.tile([P, H], BF16)  # 512*bv replicated across partitions
        wqr = w8q.rearrange("p (k o) -> p k o", o=H)
        x8r = [x8[b].rearrange("p (k t) -> p k t", t=S) for b in range(BL)]
        x8t = xtp.tile([P, KT, S], F8, tag="x8", name="x8")
        for c in range(KP):
            cs = slice(2 * c, 2 * c + 2)
            nc.sync.dma_start(out=wq_sb[:, cs, :], in_=wqr[:, cs, :])
            nc.sync.dma_start(out=x8t[:, cs, :], in_=x8r[0][:, cs, :])
        nc.sync.dma_start(out=bcol[:], in_=bcolqk)
        # wk/wv next (batch 0's K/V units need them ~10us in); batch 1's x
        # afterwards (its projections only start mid-run); wv16 last - only
        # vterm needs it
        nc.sync.dma_start(out=wk_sb[:], in_=w8k.rearrange("p (k o) -> p k o", o=H))
        nc.sync.dma_start(out=wv_sb[:], in_=w8v.rearrange("p (k o) -> p k o", o=H))
        nc.sync.dma_start(out=bvb[:], in_=bvb512)
        x8ts = [x8t]
        for b in range(1, BL):
            t8 = xtp.tile([P, KT, S], F8, tag="x8", name="x8")
            nc.sync.dma_start(out=t8[:], in_=x8r[b])
            x8ts.append(t8)
        xt16ts = []
        for b in range(BL):
            t16 = xtp.tile([P, KT, T], BF16, tag="xt16", name="xt16")
            nc.sync.dma_start(
                out=t16[:], in_=xt16[b].rearrange("p (k t) -> p k t", t=T)
            )
            xt16ts.append(t16)
        nc.sync.dma_start(out=wv16_sb[:], in_=wv16.rearrange("p (k o) -> p k o", o=H))

        def psw():
            return psp.tile([P, 1024], FP32, tag="psW", bufs=3, name="psW")

        def emit_proj_units(b):
            """Per-batch projections as a list of small thunks the scheduler
            interleaves with attention stages. Each unit fills a 2-bank psW
            tile ([0:512] bank A, rest bank B) and drains it with ONE wide
            ACT/DVE op. Returns (units, vterm_thunk, tiles)."""
            x8t, xt16t = x8ts[b], xt16ts[b]
            qt_sb = [qkv.tile([P, NQ], BF16, tag=f"qt{m}", name=f"qt{m}") for m in range(KT)]
            kt_sb = [qkv.tile([P, S], BF16, tag=f"kt{m}", name=f"kt{m}") for m in range(KT)]
            vext = [qkv.tile([P, NH * VW], BF16, tag=f"v{m}", name=f"v{m}") for m in range(KT)]
            vterm = qkv.tile([P, H], BF16, tag="vterm", name="vterm")
            vsumsE = smp.tile([CDD, NH * VW], BF16, tag="vsums", name="vsumsE")
            units = []

            def dr_chain(ps, lhs_of, rhs_of, total):
                for n0 in range(0, total, 512):
                    nlen = min(512, total - n0)
                    for c in range(KP):
                        cs = slice(2 * c, 2 * c + 2)
                        nc.tensor.matmul(
                            ps[:, n0 : n0 + nlen],
                            lhsT=lhs_of(cs),
                            rhs=rhs_of(cs, n0, nlen),
                            start=(c == 0),
                            stop=(c == KP - 1),
                            perf_mode=DR,
                        )

            # Q^T, K^T o-tile ot = 2*g4 + i; 512x bias added on the copy
            def qk_unit(wsb, dst, boff, ot, total, on_act):
                ms = slice(ot * P, (ot + 1) * P)
                ps = psw()
                dr_chain(
                    ps,
                    lambda cs: wsb[:, cs, ms],
                    lambda cs, n0, nlen: x8t[:, cs, n0 : n0 + nlen],
                    total,
                )
                bias = bcol[:, boff + ot : boff + ot + 1]
                if on_act:
                    nc.scalar.activation(
                        dst[ot][:], ps[:, 0:total], AF.Identity, bias=bias
                    )
                else:
                    nc.vector.tensor_scalar_add(dst[ot][:], ps[:, 0:total], bias)

            # V: out[t-tile, 0:768] -> vext (bf16 512x, 65-strided)
            def v_unit(mt):
                ms = slice(mt * P, (mt + 1) * P)
                ps = psw()
                dr_chain(
                    ps,
                    lambda cs: x8t[:, cs, ms],
                    lambda cs, n0, nlen: wv_sb[:, cs, n0 : n0 + nlen],
                    H,
                )
                vv = vext[mt].rearrange("p (h c) -> p h c", c=VW)
                nc.vector.tensor_tensor(
                    out=vv[:, :, 0:DH],
                    in0=ps[:, 0:H].rearrange("p (h c) -> p h c", c=DH),
                    in1=bvb.rearrange("p (h c) -> p h c", c=DH),
                    op=ALU.add,
                )
                nc.gpsimd.memset(vv[:, :, DH : DH + 1], SC)

            # per-block value sums (512x), stored with 65th col = 64*512 so
            # the notselC correction matmul also contributes 512*(9*64):
            # heads 0-7 in bank A, 8-11 in bank B, one DVE drain
            def vsum_unit():
                ps = psw()
                for n0, nh0, nh in ((0, 0, 8), (512, 8, 4)):
                    for kt in range(5):
                        rhs = vext[kt].rearrange("p (h c) -> p h c", c=VW)[
                            :, nh0 : nh0 + nh, 0:DH
                        ]
                        nc.tensor.matmul(
                            ps[0:CDD, n0 : n0 + nh * DH],
                            lhsT=G[:, 10 - 2 * kt : 20 - 2 * kt],
                            rhs=rhs,
                            start=(kt == 0),
                            stop=(kt == 4),
                        )
                vsv = vsumsE.rearrange("p (h c) -> p h c", c=VW)
                nc.vector.tensor_copy(
                    vsv[:, :, 0:DH],
                    ps[0:CDD, 0:H].rearrange("p (h c) -> p h c", c=DH),
                )
                nc.gpsimd.memset(vsv[:, :, DH : DH + 1], float(L) * SC)

            # Q first (needs only wq+x8, which DMA first), then per g4-group
            # K then V so ACT (Q) and DVE (K/V) drain concurrently
            for ot in range(2 * NHG):
                units.append(
                    lambda ot=ot: qk_unit(wq_sb, qt_sb, 0, ot, NQ, True)
                )
            for g4 in range(NHG):
                for i in range(2):
                    units.append(
                        lambda ot=2 * g4 + i: qk_unit(
                            wk_sb, kt_sb, KT, ot, S, ot % 2 == 1
                        )
                    )
                units.append(lambda mt=2 * g4: v_unit(mt))
                units.append(lambda mt=2 * g4 + 1: v_unit(mt))
            units.append(vsum_unit)

            # term-value passthrough rows in bf16 (fp8 noise would be exposed
            # verbatim in the output); bias rides a rank-1 (1/512)*bvb matmul
            def vterm_unit():
                ps = psw()
                for n0, nlen in ((0, 512), (512, 256)):
                    for kt in range(KT):
                        nc.tensor.matmul(
                            ps[:, n0 : n0 + nlen],
                            lhsT=xt16t[:, kt, :],
                            rhs=wv16_sb[:, kt, n0 : n0 + nlen],
                            start=(kt == 0),
                            stop=False,
                        )
                    nc.tensor.matmul(
                        ps[:, n0 : n0 + nlen],
                        lhsT=invrow[:],
                        rhs=bvb[0:1, n0 : n0 + nlen],
                        start=False,
                        stop=True,
                    )
                nc.scalar.activation(vterm[:], ps[:, 0:H], AF.Copy)
                nc.sync.dma_start(out=out[b][NQ:S, :], in_=vterm[:])

            return units, vterm_unit, (qt_sb, kt_sb, vext, vterm, vsumsE)

        def make_attention(b, tiles):
            """Return (scores, pv) stage emitters for batch b; the top-level
            schedule interleaves them with projection units."""
            qt_sb, kt_sb, vext, vterm, vsumsE = tiles
            obs = [
                osp.tile([P, H], BF16, tag=f"osb{j}", name=f"osb{j}")
                for j in range(5)
            ]
            ses = {}

            def emit_scores(hg, filler=None):
                # per head: one 2-bank tile holds term scores^T at [0:640]
                # and all 10 block products at [640:960]; ONE 960-wide exp.
                # All score matmuls are DoubleRow over (32 partitions, 2).
                se = [sep.tile([P, SB + 5 * L], BF16, tag=f"se{i}", name=f"se{i}") for i in range(HGS)]
                for hl in range(HGS):
                    if filler is not None:
                        filler()
                    hh = hg * HGS + hl
                    pt, r0 = hh // 2, (hh % 2) * 64
                    QTh = qt_sb[pt][r0 : r0 + 64, :]
                    KTh = kt_sb[pt][r0 : r0 + 64, :]
                    ps = psw()
                    for n0, nlen in ((0, 512), (512, 128)):
                        nc.tensor.matmul(
                            ps[:, n0 : n0 + nlen],
                            lhsT=KTh[:, NQ:S],
                            rhs=QTh[:, n0 : n0 + nlen],
                            start=True,
                            stop=True,
                        )
                    for j in range(5):
                        for half in (0, 1):
                            c = 2 * j + half
                            cs = slice(c * L, (c + 1) * L)
                            nc.tensor.matmul(
                                ps[half * 64 : half * 64 + 64, SB + j * L : SB + (j + 1) * L],
                                lhsT=KTh[:, cs],
                                rhs=QTh[:, cs],
                                start=True,
                                stop=True,
                            )
                    nc.scalar.activation(
                        se[hl][:],
                        ps[:, 0 : SB + 5 * L],
                        AF.Exp,
                        scale=ESCALE,
                    )
                ses[hg] = se

            def emit_pv(hg, filler=None):
                se = ses.pop(hg)
                for j in range(5):
                    if filler is not None:
                        filler()
                    psc = psp.tile([P, HGS * VW], FP32, tag="psC", bufs=2, name="psC", padded_shape=[P, 512])
                    hgs_v = slice(hg * HGS * VW, (hg + 1) * HGS * VW)
                    # head 0's full-height terms matmul opens the bank's one
                    # accumulation group; everything else accumulates.
                    for hl in range(HGS):
                        hh = hg * HGS + hl
                        vs = slice(hh * VW, (hh + 1) * VW)
                        nc.tensor.matmul(
                            psc[:, hl * VW : (hl + 1) * VW],
                            lhsT=se[hl][:, j * P : (j + 1) * P],
                            rhs=vext[5][:, vs],
                            start=(hl == 0),
                            stop=False,
                        )
                    # correction: one full-height K=10 matmul per j pair
                    nc.tensor.matmul(
                        psc[:, :],
                        lhsT=notselC[:, (2 * j) * L : (2 * j + 2) * L],
                        rhs=vsumsE[:, hgs_v],
                        start=False,
                        stop=False,
                    )
                    for hl in range(HGS):
                        hh = hg * HGS + hl
                        c0 = hl * VW
                        vs = slice(hh * VW, (hh + 1) * VW)
                        for half in (0, 1):
                            hs = slice(half * 64, half * 64 + 64)
                            nc.tensor.matmul(
                                psc[hs, c0 : c0 + VW],
                                lhsT=se[hl][hs, SB + j * L : SB + (j + 1) * L],
                                rhs=vext[j][hs, vs],
                                start=False,
                                stop=False,
                            )
                    # full-height +0 rank-1 whose stop closes the bank's group
                    nc.tensor.matmul(
                        psc[:, DH : DH + 1],
                        lhsT=onesrow[:],
                        rhs=zrow[:],
                        start=False,
                        stop=True,
                    )
                    zr = smp.tile([P, HGS], FP32, tag="zr", bufs=4, name="zr")
                    pscv = psc.rearrange("p (h c) -> p h c", c=VW)
                    nc.vector.reciprocal(
                        zr[:].rearrange("p (h o) -> p h o", o=1),
                        pscv[:, :, DH : DH + 1],
                    )
                    in0 = pscv[:, :, 0:DH]
                    in1 = zr[:].rearrange("p (h o) -> p h o", o=1)
                    bin0, bin1 = bass.broadcast_tensor_aps(in0, in1)
                    nc.vector.tensor_tensor(
                        out=obs[j][:, hg * HGS * DH : (hg + 1) * HGS * DH].rearrange(
                            "p (h c) -> p h c", c=DH
                        ),
                        in0=bin0,
                        in1=bin1,
                        op=ALU.mult,
                    )
                    if hg == NHG - 1:
                        nc.sync.dma_start(
                            out=out[b][j * P : (j + 1) * P, :],
                            in_=obs[j][:],
                        )

            return emit_scores, emit_pv

        def taker(q, k):
            def f():
                for _ in range(min(k, len(q))):
                    q.popleft()()
            return f

        # Alternating scores/PV pipeline: each scores stage's 4 exps
        # (ACT-heavy) overlap the neighboring PV stages' matmuls (PE-heavy);
        # batch 1's projections and both vterms fill the gaps in between.
        units0, vterm0, tiles0 = emit_proj_units(0)
        scores0, pv0 = make_attention(0, tiles0)
        q0 = deque(units0)  # 6 Q, 6 K, 6 V, 1 vsums
        if BL > 1:
            units1, vterm1, tiles1 = emit_proj_units(1)
            scores1, pv1 = make_attention(1, tiles1)
            taker(q0, 6)()  # b0 Q (only wq+x8[0] DMAs needed)
            taker(q0, 2)()  # b0 K g4=0
            scores0(0, taker(q0, 2))  # b0 V0-1, K g4=1, V2-3, K g4=2
            taker(q0, 3)()  # b0 V4-5 + vsums
            fill = deque(units1 + [vterm0, vterm1])
            pv0(0, taker(fill, 2))  # b1 Q + K start
            scores0(1, taker(fill, 1))
            pv0(1, taker(fill, 1))
            scores0(2, taker(fill, 1))  # drains the vterms
            pv0(2, taker(fill, 1))
            scores1(0, None)
            pv1(0, None)
            scores1(1, None)
            pv1(1, None)
            scores1(2, None)
            pv1(2, None)
        else:
            taker(q0, 19)()
            scores0(0, None)
            scores0(1, None)
            scores0(2, None)
            vterm0()
            pv0(0)
            pv0(1)
            pv0(2)

_CACHE = {}


def _get_program():
    if "nc" not in _CACHE:
        _CACHE["nc"] = _build_program()
    return _CACHE["nc"]


def _make_in_maps(inputs):
    f8 = ml_dtypes.float8_e4m3
    bf = ml_dtypes.bfloat16
    hs = np.asarray(inputs["hidden_states"], np.float32)
    hst = hs.transpose(0, 2, 1)  # [B, H, S]
    x8 = np.ascontiguousarray(
        (hst * SX).reshape(B, KT, P, S).transpose(0, 2, 1, 3).reshape(B, P, KT * S)
    ).astype(f8)
    xt16 = np.ascontiguousarray(
        hst[:, :, NQ:].reshape(B, KT, P, T).transpose(0, 2, 1, 3).reshape(B, P, KT * T)
    ).astype(bf)

    def prep_w(w, scale, dtype, perm=None):  # [out, in] -> W^T tiled
        wt = np.asarray(w, np.float32).T * scale  # [in, out]
        if perm is not None:
            wt = wt[:, perm]
        return np.ascontiguousarray(
            wt.reshape(KT, P, H).transpose(1, 0, 2).reshape(P, KT * H)
        ).astype(dtype)

    bq = np.asarray(inputs["bq"], np.float32) * SC
    bk = np.asarray(inputs["bk"], np.float32) * SC
    bcolqk = np.ascontiguousarray(
        np.concatenate([bq.reshape(KT, P).T, bk.reshape(KT, P).T], axis=1)
    ).astype(np.float32)
    bvb512 = np.tile(
        (np.asarray(inputs["bv"], np.float32) * SC).astype(bf)[None, :], (P, 1)
    )
    in_common = {
        "w8q": prep_w(inputs["Wq"], SWT, f8),
        "w8k": prep_w(inputs["Wk"], SWT, f8),
        "w8v": prep_w(inputs["Wv"], SWT, f8),
        "wv16": prep_w(inputs["Wv"], 1.0, bf),
        "bcolqk": bcolqk,
        "bvb512": bvb512,
    }
    return [
        {
            "x8": x8[i * BL : (i + 1) * BL],
            "xt16": xt16[i * BL : (i + 1) * BL],
            **in_common,
        }
        for i in range(NCORES)
    ]


def kernel(**inputs) -> np.ndarray:
    in_maps = _make_in_maps(inputs)
    nc = _get_program()
    res = run_bass_kernel_spmd(nc, in_maps, list(range(NCORES)))
    return np.concatenate(
        [res.results[i]["out"] for i in range(NCORES)], axis=0
    ).astype(np.float32)


# revision 66
# speedup vs baseline: 1.0029x; 1.0029x over previous
"""Trainium2 Bass kernel for nn_BertSelfAttention_79577154060613.

Block-sparse BERT self-attention (block-diagonal over 10 candidate blocks of
64 tokens + dense global columns for 128 term tokens), data-parallel over
batch across 8 NeuronCores (2 batches per core).

Key algorithmic trick: the reference multiplies scores by the mask (masked
entries become exactly 0, not -inf), so softmax gives each masked key weight
exp(0)=1. For a query in block c:
    ctx = (sum_{k in block c | terms} e^{s_k} v_k + sum_{c' != c} Vsum_c') / Z
    Z   = sum_{k in block c | terms} e^{s_k} + 9*64
where Vsum_c' are per-head, per-block sums of candidate value rows. This
turns 768-wide attention into 192-wide attention plus one small K=10 matmul
(lhsT = 1 - one_hot(c)) per query tile.

The three projections run as fp8e4 DoubleRow matmuls (two fp8 elements per
PE cell -> half the instructions at twice the rate): the host pre-scales x
by 16 and W^T by 32 (power-of-two, exact) and packs the H=768 contraction
as 3 [128 partitions, 2, .] chunk-pair 3D APs. (Scores stay bf16: walrus
rejects DoubleRow combined with PE column tiling, which the 64-partition
block-score matmuls need.) Everything downstream carries the combined 512x
scale: the exp scale folds in 2^-21 exactly, V/Vsums are stored 512x with
the softmax-denominator ones column set to 512 (and 64*512 in Vsums), so
the final reciprocal-multiply cancels the scale with zero extra
instructions. fp8 rounding noise averages
out across the 768-key softmax (<1e-3 relative) everywhere except the term
value rows, which pass through to the output verbatim - those are recomputed
in bf16 (a small [128, H] matmul) with the bias folded in as a rank-1 matmul.

PSUM layout: one [128, 1024] 2-bank tag (psW, 3 bufs) hosts every projection
and score group so each drains with ONE wide ACT/DVE op (engine init
latency ~150-185ns dominates small copies; GpSimd has no PSUM port at all).
A head's scores pack term chunks at [0:512],[512:640] and all 10 block
products at [640:960] - one 960-wide exp per head. PV accumulates in psC
(2 single-bank bufs); each bank hosts its groups opened by full-height
matmuls and closed by a rank-1 +0 whose stop ends the group. The emission
order software-pipelines the two batches: scores/PV stages alternate and
projection units of the other batch fill every gap, so exp work (ACT),
PSUM drains (ACT+DVE split) and matmuls (PE) overlap throughout.
"""

import numpy as np
import ml_dtypes

import concourse.bass as bass
import concourse.mybir as mybir
import concourse.tile as tile
from concourse import bacc
from concourse.bass_utils import run_bass_kernel_spmd

# Problem dims (hardcoded per contract)
B, CDD, L, T, H, NH = 16, 10, 64, 128, 768, 12
DH = H // NH  # 64
S = CDD * L + T  # 768
NQ = CDD * L  # 640
P = 128
NCORES = 8
BL = B // NCORES  # 2 batches per core
KT = H // P  # 6 contraction tiles
KP = KT // 2  # 3 DoubleRow chunk pairs
FP32 = mybir.dt.float32
BF16 = mybir.dt.bfloat16
F8 = mybir.dt.float8e4
AF = mybir.ActivationFunctionType
ALU = mybir.AluOpType
DR = mybir.MatmulPerfMode.DoubleRow
HGS = 4  # heads per attention group (= heads per Q/K o-tile)
NHG = NH // HGS  # 3 groups
VW = DH + 1  # value width per head incl. denominator column (65)
SX = 16.0  # fp8 scale on x
SWT = 32.0  # fp8 scale on W
SC = SX * SWT  # combined 512x scale carried through the kernel
ESCALE = 0.125 / (SC * SC)  # exp scale: 1/sqrt(DH) / 512^2 = 2^-21 (exact)
SB = 640  # block-scores base column inside a head's score tile


def _build_program():
    nc = bacc.Bacc(
        "TRN2", target_bir_lowering=False, debug=False, num_devices=NCORES
    )
    x8 = nc.dram_tensor("x8", [BL, P, KT * S], F8, kind="ExternalInput").ap()
    xt16 = nc.dram_tensor("xt16", [BL, P, KT * T], BF16, kind="ExternalInput").ap()
    w8q = nc.dram_tensor("w8q", [P, KT * H], F8, kind="ExternalInput").ap()
    w8k = nc.dram_tensor("w8k", [P, KT * H], F8, kind="ExternalInput").ap()
    w8v = nc.dram_tensor("w8v", [P, KT * H], F8, kind="ExternalInput").ap()
    wv16 = nc.dram_tensor("wv16", [P, KT * H], BF16, kind="ExternalInput").ap()
    bcolqk = nc.dram_tensor("bcolqk", [P, 2 * KT], FP32, kind="ExternalInput").ap()
    bvb512 = nc.dram_tensor("bvb512", [P, H], BF16, kind="ExternalInput").ap()
    out = nc.dram_tensor("out", [BL, S, H], BF16, kind="ExternalOutput").ap()

    with tile.TileContext(nc) as tc:
        _emit(tc, nc, x8, xt16, w8q, w8k, w8v, wv16, bcolqk, bvb512, out)
    nc.compile()
    return nc


def _emit(tc, nc, x8, xt16, w8q, w8k, w8v, wv16, bcolqk, bvb512, out):
    from collections import deque
    from contextlib import ExitStack

    ctx = ExitStack()
    with ctx:
        cpool = ctx.enter_context(tc.tile_pool(name="consts", bufs=1))
        wpool = ctx.enter_context(tc.tile_pool(name="weights", bufs=1))
        xtp = ctx.enter_context(tc.tile_pool(name="xt", bufs=2))
        qkv = ctx.enter_context(tc.tile_pool(name="qkv", bufs=2))
        sep = ctx.enter_context(tc.tile_pool(name="se", bufs=4))
        osp = ctx.enter_context(tc.tile_pool(name="osb", bufs=2))
        smp = ctx.enter_context(tc.tile_pool(name="small", bufs=2))
        psp = ctx.enter_context(tc.tile_pool(name="psum", bufs=1, space="PSUM"))

        # ---- constants ----
        onesrow = cpool.tile([1, P], BF16)  # 1.0 row (group-closer rank-1 lhsT)
        nc.gpsimd.memset(onesrow[:], 1.0)
        # tiny activation at t=0 pulls the implicit ACT table load into the
        # initial DMA wait instead of blocking the first real copy
        actwarm = cpool.tile([1, P], BF16)
        nc.scalar.activation(actwarm[:], onesrow[:], AF.Exp)
        invrow = cpool.tile([1, P], BF16)  # 1/512 row (vterm bias rank-1 lhsT)
        nc.gpsimd.memset(invrow[:], 1.0 / SC)
        zrow = cpool.tile([1, 1], BF16)  # 0.0 (group-closer rank-1 rhs)
        nc.gpsimd.memset(zrow[:], 0.0)
        # notselC[p, c*64+j] = 0 if p == c else 1  (p in 0..9)
        notselC = cpool.tile([CDD, NQ], BF16)
        nc.gpsimd.memset(notselC[:], 1.0)
        nc.gpsimd.affine_select(
            out=notselC.rearrange("p (c j) -> p c j", j=L),
            in_=notselC.rearrange("p (c j) -> p c j", j=L),
            compare_op=ALU.not_equal,
            fill=0.0,
            base=0,
            pattern=[[-1, CDD], [0, L]],
            channel_multiplier=1,
        )
        # block-membership indicator for Vsums: G[p, j] = 1 iff j-10 == p//64
        G = cpool.tile([P, 20], BF16)
        nc.gpsimd.memset(G[:], 0.0)
        nc.gpsimd.memset(G[0:64, 10:11], 1.0)
        nc.gpsimd.memset(G[64:128, 11:12], 1.0)

        # ---- weights, biases, x (all layouts host-prepared) ----
        wq_sb = wpool.tile([P, KT, H], F8, tag="wq", name="wq")
        wk_sb = wpool.tile([P, KT, H], F8, tag="wk", name="wk")
        wv_sb = wpool.tile([P, KT, H], F8, tag="wv", name="wv")
        wv16_sb = wpool.tile([P, KT, H], BF16, tag="wv16", name="wv16")
        bcol = cpool.tile([P, 2 * KT], FP32)
        bvb = cpool.tile([P, H], BF16)  # 512*bv replicated across partitions
        wqr = w8q.rearrange("p (k o) -> p k o", o=H)
        x8r = [x8[b].rearrange("p (k t) -> p k t", t=S) for b in range(BL)]
        x8t = xtp.tile([P, KT, S], F8, tag="x8", name="x8")
        for c in range(KP):
            cs = slice(2 * c, 2 * c + 2)
            nc.sync.dma_start(out=wq_sb[:, cs, :], in_=wqr[:, cs, :])
            nc.sync.dma_start(out=x8t[:, cs, :], in_=x8r[0][:, cs, :])
        nc.sync.dma_start(out=bcol[:], in_=bcolqk)
        # wk/wv next (batch 0's K/V units need them ~10us in); batch 1's x
        # afterwards (its projections only start mid-run); wv16 last - only
        # vterm needs it
        nc.sync.dma_start(out=wk_sb[:], in_=w8k.rearrange("p (k o) -> p k o", o=H))
        nc.sync.dma_start(out=wv_sb[:], in_=w8v.rearrange("p (k o) -> p k o", o=H))
        nc.sync.dma_start(out=bvb[:], in_=bvb512)
        x8ts = [x8t]
        for b in range(1, BL):
            t8 = xtp.tile([P, KT, S], F8, tag="x8", name="x8")
            nc.sync.dma_start(out=t8[:], in_=x8r[b])
            x8ts.append(t8)
        xt16ts = []
        for b in range(BL):
            t16 = xtp.tile([P, KT, T], BF16, tag="xt16", name="xt16")
            nc.sync.dma_start(
                out=t16[:], in_=xt16[b].rearrange("p (k t) -> p k t", t=T)
            )
            xt16ts.append(t16)
        nc.sync.dma_start(out=wv16_sb[:], in_=wv16.rearrange("p (k o) -> p k o", o=H))

        def psw():
            return psp.tile([P, 1024], FP32, tag="psW", bufs=3, name="psW")

        def emit_proj_units(b):
            """Per-batch projections as a list of small thunks the scheduler
            interleaves with attention stages. Each unit fills a 2-bank psW
            tile ([0:512] bank A, rest bank B) and drains it with ONE wide
            ACT/DVE op. Returns (units, vterm_thunk, tiles)."""
            x8t, xt16t = x8ts[b], xt16ts[b]
            qt_sb = [qkv.tile([P, NQ], BF16, tag=f"qt{m}", name=f"qt{m}") for m in range(KT)]
            kt_sb = [qkv.tile([P, S], BF16, tag=f"kt{m}", name=f"kt{m}") for m in range(KT)]
            vext = [qkv.tile([P, NH * VW], BF16, tag=f"v{m}", name=f"v{m}") for m in range(KT)]
            vterm = qkv.tile([P, H], BF16, tag="vterm", name="vterm")
            vsumsE = smp.tile([CDD, NH * VW], BF16, tag="vsums", name="vsumsE")
            units = []

            def dr_chain(ps, lhs_of, rhs_of, total):
                for n0 in range(0, total, 512):
                    nlen = min(512, total - n0)
                    for c in range(KP):
                        cs = slice(2 * c, 2 * c + 2)
                        nc.tensor.matmul(
                            ps[:, n0 : n0 + nlen],
                            lhsT=lhs_of(cs),
                            rhs=rhs_of(cs, n0, nlen),
                            start=(c == 0),
                            stop=(c == KP - 1),
                            perf_mode=DR,
                        )

            # Q^T, K^T o-tile ot = 2*g4 + i; 512x bias added on the copy
            def qk_unit(wsb, dst, boff, ot, total, on_act):
                ms = slice(ot * P, (ot + 1) * P)
                ps = psw()
                dr_chain(
                    ps,
                    lambda cs: wsb[:, cs, ms],
                    lambda cs, n0, nlen: x8t[:, cs, n0 : n0 + nlen],
                    total,
                )
                bias = bcol[:, boff + ot : boff + ot + 1]
                if on_act:
                    nc.scalar.activation(
                        dst[ot][:], ps[:, 0:total], AF.Identity, bias=bias
                    )
                else:
                    nc.vector.tensor_scalar_add(dst[ot][:], ps[:, 0:total], bias)

            # V: out[t-tile, 0:768] -> vext (bf16 512x, 65-strided)
            def v_unit(mt):
                ms = slice(mt * P, (mt + 1) * P)
                ps = psw()
                dr_chain(
                    ps,
                    lambda cs: x8t[:, cs, ms],
                    lambda cs, n0, nlen: wv_sb[:, cs, n0 : n0 + nlen],
                    H,
                )
                vv = vext[mt].rearrange("p (h c) -> p h c", c=VW)
                nc.vector.tensor_tensor(
                    out=vv[:, :, 0:DH],
                    in0=ps[:, 0:H].rearrange("p (h c) -> p h c", c=DH),
                    in1=bvb.rearrange("p (h c) -> p h c", c=DH),
                    op=ALU.add,
                )
                nc.gpsimd.memset(vv[:, :, DH : DH + 1], SC)

            # per-block value sums (512x), stored with 65th col = 64*512 so
            # the notselC correction matmul also contributes 512*(9*64):
            # heads 0-7 in bank A, 8-11 in bank B, one DVE drain
            def vsum_unit():
                ps = psw()
                for n0, nh0, nh in ((0, 0, 8), (512, 8, 4)):
                    for kt in range(5):
                        rhs = vext[kt].rearrange("p (h c) -> p h c", c=VW)[
                            :, nh0 : nh0 + nh, 0:DH
                        ]
                        nc.tensor.matmul(
                            ps[0:CDD, n0 : n0 + nh * DH],
                            lhsT=G[:, 10 - 2 * kt : 20 - 2 * kt],
                            rhs=rhs,
                            start=(kt == 0),
                            stop=(kt == 4),
                        )
                vsv = vsumsE.rearrange("p (h c) -> p h c", c=VW)
                nc.vector.tensor_copy(
                    vsv[:, :, 0:DH],
                    ps[0:CDD, 0:H].rearrange("p (h c) -> p h c", c=DH),
                )
                nc.gpsimd.memset(vsv[:, :, DH : DH + 1], float(L) * SC)

            # Q first (needs only wq+x8, which DMA first), then per g4-group
            # K then V so ACT (Q) and DVE (K/V) drain concurrently
            for ot in range(2 * NHG):
                units.append(
                    lambda ot=ot: qk_unit(wq_sb, qt_sb, 0, ot, NQ, True)
                )
            for g4 in range(NHG):
                for i in range(2):
                    units.append(
                        lambda ot=2 * g4 + i: qk_unit(
                            wk_sb, kt_sb, KT, ot, S, ot % 2 == 1
                        )
                    )
                units.append(lambda mt=2 * g4: v_unit(mt))
                units.append(lambda mt=2 * g4 + 1: v_unit(mt))
            units.append(vsum_unit)

            # term-value passthrough rows in bf16 (fp8 noise would be exposed
            # verbatim in the output); bias rides a rank-1 (1/512)*bvb matmul
            def vterm_unit():
                ps = psw()
                for n0, nlen in ((0, 512), (512, 256)):
                    for kt in range(KT):
                        nc.tensor.matmul(
                            ps[:, n0 : n0 + nlen],
                            lhsT=xt16t[:, kt, :],
                            rhs=wv16_sb[:, kt, n0 : n0 + nlen],
                            start=(kt == 0),
                            stop=False,
                        )
                    nc.tensor.matmul(
                        ps[:, n0 : n0 + nlen],
                        lhsT=invrow[:],
                        rhs=bvb[0:1, n0 : n0 + nlen],
                        start=False,
                        stop=True,
                    )
                nc.scalar.activation(vterm[:], ps[:, 0:H], AF.Copy)
                nc.sync.dma_start(out=out[b][NQ:S, :], in_=vterm[:])

            return units, vterm_unit, (qt_sb, kt_sb, vext, vterm, vsumsE)

        def make_attention(b, tiles):
            """Return (scores, pv) stage emitters for batch b; the top-level
            schedule interleaves them with projection units."""
            qt_sb, kt_sb, vext, vterm, vsumsE = tiles
            obs = [
                osp.tile([P, H], BF16, tag=f"osb{j}", name=f"osb{j}")
                for j in range(5)
            ]
            ses = {}

            def emit_scores(hg, filler=None):
                # per head: one 2-bank tile holds term scores^T at [0:640]
                # and all 10 block products at [640:960]; ONE 960-wide exp.
                # All score matmuls are DoubleRow over (32 partitions, 2).
                se = [sep.tile([P, SB + 5 * L], BF16, tag=f"se{i}", name=f"se{i}") for i in range(HGS)]
                for hl in range(HGS):
                    if filler is not None:
                        filler()
                    hh = hg * HGS + hl
                    pt, r0 = hh // 2, (hh % 2) * 64
                    QTh = qt_sb[pt][r0 : r0 + 64, :]
                    KTh = kt_sb[pt][r0 : r0 + 64, :]
                    ps = psw()
                    for n0, nlen in ((0, 512), (512, 128)):
                        nc.tensor.matmul(
                            ps[:, n0 : n0 + nlen],
                            lhsT=KTh[:, NQ:S],
                            rhs=QTh[:, n0 : n0 + nlen],
                            start=True,
                            stop=True,
                        )
                    for j in range(5):
                        for half in (0, 1):
                            c = 2 * j + half
                            cs = slice(c * L, (c + 1) * L)
                            nc.tensor.matmul(
                                ps[half * 64 : half * 64 + 64, SB + j * L : SB + (j + 1) * L],
                                lhsT=KTh[:, cs],
                                rhs=QTh[:, cs],
                                start=True,
                                stop=True,
                            )
                    nc.scalar.activation(
                        se[hl][:],
                        ps[:, 0 : SB + 5 * L],
                        AF.Exp,
                        scale=ESCALE,
                    )
                ses[hg] = se

            def emit_pv(hg, filler=None, tailmode=False):
                se = ses.pop(hg)
                for j in range(5):
                    if filler is not None:
                        filler()
                    if tailmode and j % 2 == 1:
                        # final stage: psW is quiescent, borrow its banks so
                        # more PV groups are in flight than psC's 2 bufs
                        psc = psw()[:, 0 : HGS * VW]
                    else:
                        psc = psp.tile([P, HGS * VW], FP32, tag="psC", bufs=2, name="psC", padded_shape=[P, 512])
                    hgs_v = slice(hg * HGS * VW, (hg + 1) * HGS * VW)
                    # head 0's full-height terms matmul opens the bank's one
                    # accumulation group; everything else accumulates.
                    for hl in range(HGS):
                        hh = hg * HGS + hl
                        vs = slice(hh * VW, (hh + 1) * VW)
                        nc.tensor.matmul(
                            psc[:, hl * VW : (hl + 1) * VW],
                            lhsT=se[hl][:, j * P : (j + 1) * P],
                            rhs=vext[5][:, vs],
                            start=(hl == 0),
                            stop=False,
                        )
                    # correction: one full-height K=10 matmul per j pair
                    nc.tensor.matmul(
                        psc[:, :],
                        lhsT=notselC[:, (2 * j) * L : (2 * j + 2) * L],
                        rhs=vsumsE[:, hgs_v],
                        start=False,
                        stop=False,
                    )
                    for hl in range(HGS):
                        hh = hg * HGS + hl
                        c0 = hl * VW
                        vs = slice(hh * VW, (hh + 1) * VW)
                        for half in (0, 1):
                            hs = slice(half * 64, half * 64 + 64)
                            nc.tensor.matmul(
                                psc[hs, c0 : c0 + VW],
                                lhsT=se[hl][hs, SB + j * L : SB + (j + 1) * L],
                                rhs=vext[j][hs, vs],
                                start=False,
                                stop=False,
                            )
                    # full-height +0 rank-1 whose stop closes the bank's group
                    nc.tensor.matmul(
                        psc[:, DH : DH + 1],
                        lhsT=onesrow[:],
                        rhs=zrow[:],
                        start=False,
                        stop=True,
                    )
                    zr = smp.tile([P, HGS], FP32, tag="zr", bufs=4, name="zr")
                    pscv = psc.rearrange("p (h c) -> p h c", c=VW)
                    nc.vector.reciprocal(
                        zr[:].rearrange("p (h o) -> p h o", o=1),
                        pscv[:, :, DH : DH + 1],
                    )
                    in0 = pscv[:, :, 0:DH]
                    in1 = zr[:].rearrange("p (h o) -> p h o", o=1)
                    bin0, bin1 = bass.broadcast_tensor_aps(in0, in1)
                    nc.vector.tensor_tensor(
                        out=obs[j][:, hg * HGS * DH : (hg + 1) * HGS * DH].rearrange(
                            "p (h c) -> p h c", c=DH
                        ),
                        in0=bin0,
                        in1=bin1,
                        op=ALU.mult,
                    )
                    if hg == NHG - 1:
                        nc.sync.dma_start(
                            out=out[b][j * P : (j + 1) * P, :],
                            in_=obs[j][:],
                        )

            return emit_scores, emit_pv

        def taker(q, k):
            def f():
                for _ in range(min(k, len(q))):
                    q.popleft()()
            return f

        # Alternating scores/PV pipeline: each scores stage's 4 exps
        # (ACT-heavy) overlap the neighboring PV stages' matmuls (PE-heavy);
        # batch 1's projections and both vterms fill the gaps in between.
        units0, vterm0, tiles0 = emit_proj_units(0)
        scores0, pv0 = make_attention(0, tiles0)
        q0 = deque(units0)  # 6 Q, 6 K, 6 V, 1 vsums
        if BL > 1:
            units1, vterm1, tiles1 = emit_proj_units(1)
            scores1, pv1 = make_attention(1, tiles1)
            taker(q0, 6)()  # b0 Q (only wq+x8[0] DMAs needed)
            taker(q0, 2)()  # b0 K g4=0
            scores0(0, taker(q0, 2))  # b0 V0-1, K g4=1, V2-3, K g4=2
            taker(q0, 3)()  # b0 V4-5 + vsums
            fill = deque(units1 + [vterm0, vterm1])
            pv0(0, taker(fill, 2))  # b1 Q + K start
            scores0(1, taker(fill, 1))
            pv0(1, taker(fill, 1))
            scores0(2, taker(fill, 1))  # drains the vterms
            pv0(2, taker(fill, 1))
            scores1(0, None)
            pv1(0, None)
            scores1(1, None)
            pv1(1, None)
            scores1(2, None)
            pv1(2, None, tailmode=True)
        else:
            taker(q0, 19)()
            scores0(0, None)
            scores0(1, None)
            scores0(2, None)
            vterm0()
            pv0(0)
            pv0(1)
            pv0(2)

_CACHE = {}


def _get_program():
    if "nc" not in _CACHE:
        _CACHE["nc"] = _build_program()
    return _CACHE["nc"]


def _make_in_maps(inputs):
    f8 = ml_dtypes.float8_e4m3
    bf = ml_dtypes.bfloat16
    hs = np.asarray(inputs["hidden_states"], np.float32)
    hst = hs.transpose(0, 2, 1)  # [B, H, S]
    x8 = np.ascontiguousarray(
        (hst * SX).reshape(B, KT, P, S).transpose(0, 2, 1, 3).reshape(B, P, KT * S)
    ).astype(f8)
    xt16 = np.ascontiguousarray(
        hst[:, :, NQ:].reshape(B, KT, P, T).transpose(0, 2, 1, 3).reshape(B, P, KT * T)
    ).astype(bf)

    def prep_w(w, scale, dtype, perm=None):  # [out, in] -> W^T tiled
        wt = np.asarray(w, np.float32).T * scale  # [in, out]
        if perm is not None:
            wt = wt[:, perm]
        return np.ascontiguousarray(
            wt.reshape(KT, P, H).transpose(1, 0, 2).reshape(P, KT * H)
        ).astype(dtype)

    bq = np.asarray(inputs["bq"], np.float32) * SC
    bk = np.asarray(inputs["bk"], np.float32) * SC
    bcolqk = np.ascontiguousarray(
        np.concatenate([bq.reshape(KT, P).T, bk.reshape(KT, P).T], axis=1)
    ).astype(np.float32)
    bvb512 = np.tile(
        (np.asarray(inputs["bv"], np.float32) * SC).astype(bf)[None, :], (P, 1)
    )
    in_common = {
        "w8q": prep_w(inputs["Wq"], SWT, f8),
        "w8k": prep_w(inputs["Wk"], SWT, f8),
        "w8v": prep_w(inputs["Wv"], SWT, f8),
        "wv16": prep_w(inputs["Wv"], 1.0, bf),
        "bcolqk": bcolqk,
        "bvb512": bvb512,
    }
    return [
        {
            "x8": x8[i * BL : (i + 1) * BL],
            "xt16": xt16[i * BL : (i + 1) * BL],
            **in_common,
        }
        for i in range(NCORES)
    ]


def kernel(**inputs) -> np.ndarray:
    in_maps = _make_in_maps(inputs)
    nc = _get_program()
    res = run_bass_kernel_spmd(nc, in_maps, list(range(NCORES)))
    return np.concatenate(
        [res.results[i]["out"] for i in range(NCORES)], axis=0
    ).astype(np.float32)


# revision 67
# speedup vs baseline: 1.0100x; 1.0071x over previous
"""Trainium2 Bass kernel for nn_BertSelfAttention_79577154060613.

Block-sparse BERT self-attention (block-diagonal over 10 candidate blocks of
64 tokens + dense global columns for 128 term tokens), data-parallel over
batch across 8 NeuronCores (2 batches per core).

Key algorithmic trick: the reference multiplies scores by the mask (masked
entries become exactly 0, not -inf), so softmax gives each masked key weight
exp(0)=1. For a query in block c:
    ctx = (sum_{k in block c | terms} e^{s_k} v_k + sum_{c' != c} Vsum_c') / Z
    Z   = sum_{k in block c | terms} e^{s_k} + 9*64
where Vsum_c' are per-head, per-block sums of candidate value rows. This
turns 768-wide attention into 192-wide attention plus one small K=10 matmul
(lhsT = 1 - one_hot(c)) per query tile.

The three projections run as fp8e4 DoubleRow matmuls (two fp8 elements per
PE cell -> half the instructions at twice the rate): the host pre-scales x
by 16 and W^T by 32 (power-of-two, exact) and packs the H=768 contraction
as 3 [128 partitions, 2, .] chunk-pair 3D APs. (Scores stay bf16: walrus
rejects DoubleRow combined with PE column tiling, which the 64-partition
block-score matmuls need.) Everything downstream carries the combined 512x
scale: the exp scale folds in 2^-21 exactly, V/Vsums are stored 512x with
the softmax-denominator ones column set to 512 (and 64*512 in Vsums), so
the final reciprocal-multiply cancels the scale with zero extra
instructions. fp8 rounding noise averages
out across the 768-key softmax (<1e-3 relative) everywhere except the term
value rows, which pass through to the output verbatim - those are recomputed
in bf16 (a small [128, H] matmul) with the bias folded in as a rank-1 matmul.

PSUM layout: one [128, 1024] 2-bank tag (psW, 3 bufs) hosts every projection
and score group so each drains with ONE wide ACT/DVE op (engine init
latency ~150-185ns dominates small copies; GpSimd has no PSUM port at all).
A head's scores pack term chunks at [0:512],[512:640] and all 10 block
products at [640:960] - one 960-wide exp per head. PV accumulates in psC
(2 single-bank bufs); each bank hosts its groups opened by full-height
matmuls and closed by a rank-1 +0 whose stop ends the group. The emission
order software-pipelines the two batches: scores/PV stages alternate and
projection units of the other batch fill every gap, so exp work (ACT),
PSUM drains (ACT+DVE split) and matmuls (PE) overlap throughout.
"""

import numpy as np
import ml_dtypes

import concourse.bass as bass
import concourse.mybir as mybir
import concourse.tile as tile
from concourse import bacc
from concourse.bass_utils import run_bass_kernel_spmd

# Problem dims (hardcoded per contract)
B, CDD, L, T, H, NH = 16, 10, 64, 128, 768, 12
DH = H // NH  # 64
S = CDD * L + T  # 768
NQ = CDD * L  # 640
P = 128
NCORES = 8
BL = B // NCORES  # 2 batches per core
KT = H // P  # 6 contraction tiles
KP = KT // 2  # 3 DoubleRow chunk pairs
FP32 = mybir.dt.float32
BF16 = mybir.dt.bfloat16
F8 = mybir.dt.float8e4
AF = mybir.ActivationFunctionType
ALU = mybir.AluOpType
DR = mybir.MatmulPerfMode.DoubleRow
HGS = 4  # heads per attention group (= heads per Q/K o-tile)
NHG = NH // HGS  # 3 groups
VW = DH + 1  # value width per head incl. denominator column (65)
SX = 16.0  # fp8 scale on x
SWT = 32.0  # fp8 scale on W
SC = SX * SWT  # combined 512x scale carried through the kernel
ESCALE = 0.125 / (SC * SC)  # exp scale: 1/sqrt(DH) / 512^2 = 2^-21 (exact)
SB = 640  # block-scores base column inside a head's score tile


def _build_program():
    nc = bacc.Bacc(
        "TRN2", target_bir_lowering=False, debug=False, num_devices=NCORES
    )
    x8 = nc.dram_tensor("x8", [BL, P, KT * S], F8, kind="ExternalInput").ap()
    xt16 = nc.dram_tensor("xt16", [BL, P, KT * T], BF16, kind="ExternalInput").ap()
    w8q = nc.dram_tensor("w8q", [P, KT * H], F8, kind="ExternalInput").ap()
    w8k = nc.dram_tensor("w8k", [P, KT * H], F8, kind="ExternalInput").ap()
    w8v = nc.dram_tensor("w8v", [P, KT * H], F8, kind="ExternalInput").ap()
    wv16 = nc.dram_tensor("wv16", [P, KT * H], BF16, kind="ExternalInput").ap()
    bcolqk = nc.dram_tensor("bcolqk", [P, 2 * KT], FP32, kind="ExternalInput").ap()
    bvb512 = nc.dram_tensor("bvb512", [P, H], BF16, kind="ExternalInput").ap()
    out = nc.dram_tensor("out", [BL, S, H], BF16, kind="ExternalOutput").ap()

    with tile.TileContext(nc) as tc:
        _emit(tc, nc, x8, xt16, w8q, w8k, w8v, wv16, bcolqk, bvb512, out)
    nc.compile()
    return nc


def _emit(tc, nc, x8, xt16, w8q, w8k, w8v, wv16, bcolqk, bvb512, out):
    from collections import deque
    from contextlib import ExitStack

    ctx = ExitStack()
    with ctx:
        cpool = ctx.enter_context(tc.tile_pool(name="consts", bufs=1))
        wpool = ctx.enter_context(tc.tile_pool(name="weights", bufs=1))
        xtp = ctx.enter_context(tc.tile_pool(name="xt", bufs=2))
        qkv = ctx.enter_context(tc.tile_pool(name="qkv", bufs=2))
        sep = ctx.enter_context(tc.tile_pool(name="se", bufs=4))
        osp = ctx.enter_context(tc.tile_pool(name="osb", bufs=2))
        smp = ctx.enter_context(tc.tile_pool(name="small", bufs=2))
        psp = ctx.enter_context(tc.tile_pool(name="psum", bufs=1, space="PSUM"))

        # ---- constants ----
        onesrow = cpool.tile([1, P], BF16)  # 1.0 row (group-closer rank-1 lhsT)
        nc.gpsimd.memset(onesrow[:], 1.0)
        # tiny activation at t=0 pulls the implicit ACT table load into the
        # initial DMA wait instead of blocking the first real copy
        actwarm = cpool.tile([1, P], BF16)
        nc.scalar.activation(actwarm[:], onesrow[:], AF.Exp)
        invrow = cpool.tile([1, P], BF16)  # 1/512 row (vterm bias rank-1 lhsT)
        nc.gpsimd.memset(invrow[:], 1.0 / SC)
        zrow = cpool.tile([1, 1], BF16)  # 0.0 (group-closer rank-1 rhs)
        nc.gpsimd.memset(zrow[:], 0.0)
        # notselC[p, c*64+j] = 0 if p == c else 1  (p in 0..9)
        notselC = cpool.tile([CDD, NQ], BF16)
        nc.gpsimd.memset(notselC[:], 1.0)
        nc.gpsimd.affine_select(
            out=notselC.rearrange("p (c j) -> p c j", j=L),
            in_=notselC.rearrange("p (c j) -> p c j", j=L),
            compare_op=ALU.not_equal,
            fill=0.0,
            base=0,
            pattern=[[-1, CDD], [0, L]],
            channel_multiplier=1,
        )
        # block-membership indicator for Vsums: G[p, j] = 1 iff j-10 == p//64
        G = cpool.tile([P, 20], BF16)
        nc.gpsimd.memset(G[:], 0.0)
        nc.gpsimd.memset(G[0:64, 10:11], 1.0)
        nc.gpsimd.memset(G[64:128, 11:12], 1.0)

        # ---- weights, biases, x (all layouts host-prepared) ----
        wq_sb = wpool.tile([P, KT, H], F8, tag="wq", name="wq")
        wk_sb = wpool.tile([P, KT, H], F8, tag="wk", name="wk")
        wv_sb = wpool.tile([P, KT, H], F8, tag="wv", name="wv")
        wv16_sb = wpool.tile([P, KT, H], BF16, tag="wv16", name="wv16")
        bcol = cpool.tile([P, 2 * KT], FP32)
        bvb = cpool.tile([P, H], BF16)  # 512*bv replicated across partitions
        wqr = w8q.rearrange("p (k o) -> p k o", o=H)
        x8r = [x8[b].rearrange("p (k t) -> p k t", t=S) for b in range(BL)]
        x8t = xtp.tile([P, KT, S], F8, tag="x8", name="x8")
        for c in range(KP):
            cs = slice(2 * c, 2 * c + 2)
            nc.sync.dma_start(out=wq_sb[:, cs, :], in_=wqr[:, cs, :])
            nc.sync.dma_start(out=x8t[:, cs, :], in_=x8r[0][:, cs, :])
        nc.sync.dma_start(out=bcol[:], in_=bcolqk)
        # wk/wv next (batch 0's K/V units need them ~10us in); batch 1's x
        # afterwards (its projections only start mid-run); wv16 last - only
        # vterm needs it
        nc.sync.dma_start(out=wk_sb[:], in_=w8k.rearrange("p (k o) -> p k o", o=H))
        nc.sync.dma_start(out=wv_sb[:], in_=w8v.rearrange("p (k o) -> p k o", o=H))
        nc.sync.dma_start(out=bvb[:], in_=bvb512)
        x8ts = [x8t]
        for b in range(1, BL):
            t8 = xtp.tile([P, KT, S], F8, tag="x8", name="x8")
            nc.sync.dma_start(out=t8[:], in_=x8r[b])
            x8ts.append(t8)
        xt16ts = []
        for b in range(BL):
            t16 = xtp.tile([P, KT, T], BF16, tag="xt16", name="xt16")
            nc.sync.dma_start(
                out=t16[:], in_=xt16[b].rearrange("p (k t) -> p k t", t=T)
            )
            xt16ts.append(t16)
        nc.sync.dma_start(out=wv16_sb[:], in_=wv16.rearrange("p (k o) -> p k o", o=H))

        def psw():
            return psp.tile([P, 1024], FP32, tag="psW", bufs=3, name="psW")

        def emit_proj_units(b):
            """Per-batch projections as a list of small thunks the scheduler
            interleaves with attention stages. Each unit fills a 2-bank psW
            tile ([0:512] bank A, rest bank B) and drains it with ONE wide
            ACT/DVE op. Returns (units, vterm_thunk, tiles)."""
            x8t, xt16t = x8ts[b], xt16ts[b]
            qt_sb = [qkv.tile([P, NQ], BF16, tag=f"qt{m}", name=f"qt{m}") for m in range(KT)]
            kt_sb = [qkv.tile([P, S], BF16, tag=f"kt{m}", name=f"kt{m}") for m in range(KT)]
            vext = [qkv.tile([P, NH * VW], BF16, tag=f"v{m}", name=f"v{m}") for m in range(KT)]
            vterm = qkv.tile([P, H], BF16, tag="vterm", name="vterm")
            vsumsE = smp.tile([CDD, NH * VW], BF16, tag="vsums", name="vsumsE")
            units = []

            def dr_chain(ps, lhs_of, rhs_of, total):
                for n0 in range(0, total, 512):
                    nlen = min(512, total - n0)
                    for c in range(KP):
                        cs = slice(2 * c, 2 * c + 2)
                        nc.tensor.matmul(
                            ps[:, n0 : n0 + nlen],
                            lhsT=lhs_of(cs),
                            rhs=rhs_of(cs, n0, nlen),
                            start=(c == 0),
                            stop=(c == KP - 1),
                            perf_mode=DR,
                        )

            # Q^T, K^T o-tile ot = 2*g4 + i; 512x bias added on the copy
            def qk_unit(wsb, dst, boff, ot, total, on_act):
                ms = slice(ot * P, (ot + 1) * P)
                ps = psw()
                dr_chain(
                    ps,
                    lambda cs: wsb[:, cs, ms],
                    lambda cs, n0, nlen: x8t[:, cs, n0 : n0 + nlen],
                    total,
                )
                bias = bcol[:, boff + ot : boff + ot + 1]
                if on_act:
                    nc.scalar.activation(
                        dst[ot][:], ps[:, 0:total], AF.Identity, bias=bias
                    )
                else:
                    nc.vector.tensor_scalar_add(dst[ot][:], ps[:, 0:total], bias)

            # V: out[t-tile, 0:768] -> vext (bf16 512x, 65-strided)
            def v_unit(mt):
                ms = slice(mt * P, (mt + 1) * P)
                ps = psw()
                dr_chain(
                    ps,
                    lambda cs: x8t[:, cs, ms],
                    lambda cs, n0, nlen: wv_sb[:, cs, n0 : n0 + nlen],
                    H,
                )
                vv = vext[mt].rearrange("p (h c) -> p h c", c=VW)
                nc.vector.tensor_tensor(
                    out=vv[:, :, 0:DH],
                    in0=ps[:, 0:H].rearrange("p (h c) -> p h c", c=DH),
                    in1=bvb.rearrange("p (h c) -> p h c", c=DH),
                    op=ALU.add,
                )
                nc.gpsimd.memset(vv[:, :, DH : DH + 1], SC)

            # per-block value sums (512x), stored with 65th col = 64*512 so
            # the notselC correction matmul also contributes 512*(9*64):
            # heads 0-7 in bank A, 8-11 in bank B, one DVE drain
            def vsum_unit():
                ps = psw()
                for n0, nh0, nh in ((0, 0, 8), (512, 8, 4)):
                    for kt in range(5):
                        rhs = vext[kt].rearrange("p (h c) -> p h c", c=VW)[
                            :, nh0 : nh0 + nh, 0:DH
                        ]
                        nc.tensor.matmul(
                            ps[0:CDD, n0 : n0 + nh * DH],
                            lhsT=G[:, 10 - 2 * kt : 20 - 2 * kt],
                            rhs=rhs,
                            start=(kt == 0),
                            stop=(kt == 4),
                        )
                vsv = vsumsE.rearrange("p (h c) -> p h c", c=VW)
                nc.vector.tensor_copy(
                    vsv[:, :, 0:DH],
                    ps[0:CDD, 0:H].rearrange("p (h c) -> p h c", c=DH),
                )
                nc.gpsimd.memset(vsv[:, :, DH : DH + 1], float(L) * SC)

            # Q first (needs only wq+x8, which DMA first), then per g4-group
            # K then V so ACT (Q) and DVE (K/V) drain concurrently
            for ot in range(2 * NHG):
                units.append(
                    lambda ot=ot: qk_unit(wq_sb, qt_sb, 0, ot, NQ, True)
                )
            for g4 in range(NHG):
                for i in range(2):
                    units.append(
                        lambda ot=2 * g4 + i: qk_unit(
                            wk_sb, kt_sb, KT, ot, S, ot % 2 == 1
                        )
                    )
                units.append(lambda mt=2 * g4: v_unit(mt))
                units.append(lambda mt=2 * g4 + 1: v_unit(mt))
            units.append(vsum_unit)

            # term-value passthrough rows in bf16 (fp8 noise would be exposed
            # verbatim in the output); bias rides a rank-1 (1/512)*bvb matmul
            def vterm_unit():
                ps = psw()
                for n0, nlen in ((0, 512), (512, 256)):
                    for kt in range(KT):
                        nc.tensor.matmul(
                            ps[:, n0 : n0 + nlen],
                            lhsT=xt16t[:, kt, :],
                            rhs=wv16_sb[:, kt, n0 : n0 + nlen],
                            start=(kt == 0),
                            stop=False,
                        )
                    nc.tensor.matmul(
                        ps[:, n0 : n0 + nlen],
                        lhsT=invrow[:],
                        rhs=bvb[0:1, n0 : n0 + nlen],
                        start=False,
                        stop=True,
                    )
                nc.scalar.activation(vterm[:], ps[:, 0:H], AF.Copy)
                nc.sync.dma_start(out=out[b][NQ:S, :], in_=vterm[:])

            return units, vterm_unit, (qt_sb, kt_sb, vext, vterm, vsumsE)

        def make_attention(b, tiles):
            """Return (scores, pv) stage emitters for batch b; the top-level
            schedule interleaves them with projection units."""
            qt_sb, kt_sb, vext, vterm, vsumsE = tiles
            obs = [
                osp.tile([P, H], BF16, tag=f"osb{j}", name=f"osb{j}")
                for j in range(5)
            ]
            ses = {}

            def emit_scores(hg, filler=None):
                # per head: one 2-bank tile holds term scores^T at [0:640]
                # and all 10 block products at [640:960]; ONE 960-wide exp.
                # All score matmuls are DoubleRow over (32 partitions, 2).
                se = [sep.tile([P, SB + 5 * L], BF16, tag=f"se{i}", name=f"se{i}") for i in range(HGS)]
                for hl in range(HGS):
                    if filler is not None:
                        filler()
                    hh = hg * HGS + hl
                    pt, r0 = hh // 2, (hh % 2) * 64
                    QTh = qt_sb[pt][r0 : r0 + 64, :]
                    KTh = kt_sb[pt][r0 : r0 + 64, :]
                    ps = psw()
                    for n0, nlen in ((0, 512), (512, 128)):
                        nc.tensor.matmul(
                            ps[:, n0 : n0 + nlen],
                            lhsT=KTh[:, NQ:S],
                            rhs=QTh[:, n0 : n0 + nlen],
                            start=True,
                            stop=True,
                        )
                    for j in range(5):
                        for half in (0, 1):
                            c = 2 * j + half
                            cs = slice(c * L, (c + 1) * L)
                            nc.tensor.matmul(
                                ps[half * 64 : half * 64 + 64, SB + j * L : SB + (j + 1) * L],
                                lhsT=KTh[:, cs],
                                rhs=QTh[:, cs],
                                start=True,
                                stop=True,
                            )
                    nc.scalar.activation(
                        se[hl][:],
                        ps[:, 0 : SB + 5 * L],
                        AF.Exp,
                        scale=ESCALE,
                    )
                ses[hg] = se

            def emit_pv(hg, filler=None, tailmode=False):
                se = ses.pop(hg)
                for j in range(5):
                    if filler is not None:
                        filler()
                    if tailmode and j % 2 == 1:
                        # final stage: psW is quiescent, borrow its banks so
                        # more PV groups are in flight than psC's 2 bufs
                        psc = psw()[:, 0 : HGS * VW]
                    else:
                        psc = psp.tile([P, HGS * VW], FP32, tag="psC", bufs=2, name="psC", padded_shape=[P, 512])
                    hgs_v = slice(hg * HGS * VW, (hg + 1) * HGS * VW)
                    # head 0's full-height terms matmul opens the bank's one
                    # accumulation group; everything else accumulates.
                    for hl in range(HGS):
                        hh = hg * HGS + hl
                        vs = slice(hh * VW, (hh + 1) * VW)
                        nc.tensor.matmul(
                            psc[:, hl * VW : (hl + 1) * VW],
                            lhsT=se[hl][:, j * P : (j + 1) * P],
                            rhs=vext[5][:, vs],
                            start=(hl == 0),
                            stop=False,
                        )
                    # correction: one full-height K=10 matmul per j pair
                    nc.tensor.matmul(
                        psc[:, :],
                        lhsT=notselC[:, (2 * j) * L : (2 * j + 2) * L],
                        rhs=vsumsE[:, hgs_v],
                        start=False,
                        stop=False,
                    )
                    for hl in range(HGS):
                        hh = hg * HGS + hl
                        c0 = hl * VW
                        vs = slice(hh * VW, (hh + 1) * VW)
                        for half in (0, 1):
                            hs = slice(half * 64, half * 64 + 64)
                            nc.tensor.matmul(
                                psc[hs, c0 : c0 + VW],
                                lhsT=se[hl][hs, SB + j * L : SB + (j + 1) * L],
                                rhs=vext[j][hs, vs],
                                start=False,
                                stop=False,
                            )
                    # full-height +0 rank-1 whose stop closes the bank's group
                    nc.tensor.matmul(
                        psc[:, DH : DH + 1],
                        lhsT=onesrow[:],
                        rhs=zrow[:],
                        start=False,
                        stop=True,
                    )
                    zr = smp.tile([P, HGS], FP32, tag="zr", bufs=4, name="zr")
                    pscv = psc.rearrange("p (h c) -> p h c", c=VW)
                    nc.vector.reciprocal(
                        zr[:].rearrange("p (h o) -> p h o", o=1),
                        pscv[:, :, DH : DH + 1],
                    )
                    in0 = pscv[:, :, 0:DH]
                    in1 = zr[:].rearrange("p (h o) -> p h o", o=1)
                    bin0, bin1 = bass.broadcast_tensor_aps(in0, in1)
                    nc.vector.tensor_tensor(
                        out=obs[j][:, hg * HGS * DH : (hg + 1) * HGS * DH].rearrange(
                            "p (h c) -> p h c", c=DH
                        ),
                        in0=bin0,
                        in1=bin1,
                        op=ALU.mult,
                    )
                    if hg == NHG - 1:
                        if b == BL - 1:
                            # tail: ship the first 2 head-groups' columns as
                            # soon as this j's last mult lands; only a third
                            # of the bytes trail the final compute
                            nc.sync.dma_start(
                                out=out[b][j * P : (j + 1) * P, 0 : 2 * HGS * DH],
                                in_=obs[j][:, 0 : 2 * HGS * DH],
                            )
                            nc.sync.dma_start(
                                out=out[b][j * P : (j + 1) * P, 2 * HGS * DH : H],
                                in_=obs[j][:, 2 * HGS * DH : H],
                            )
                        else:
                            nc.sync.dma_start(
                                out=out[b][j * P : (j + 1) * P, :],
                                in_=obs[j][:],
                            )

            return emit_scores, emit_pv

        def taker(q, k):
            def f():
                for _ in range(min(k, len(q))):
                    q.popleft()()
            return f

        # Alternating scores/PV pipeline: each scores stage's 4 exps
        # (ACT-heavy) overlap the neighboring PV stages' matmuls (PE-heavy);
        # batch 1's projections and both vterms fill the gaps in between.
        units0, vterm0, tiles0 = emit_proj_units(0)
        scores0, pv0 = make_attention(0, tiles0)
        q0 = deque(units0)  # 6 Q, 6 K, 6 V, 1 vsums
        if BL > 1:
            units1, vterm1, tiles1 = emit_proj_units(1)
            scores1, pv1 = make_attention(1, tiles1)
            taker(q0, 6)()  # b0 Q (only wq+x8[0] DMAs needed)
            taker(q0, 2)()  # b0 K g4=0
            scores0(0, taker(q0, 2))  # b0 V0-1, K g4=1, V2-3, K g4=2
            taker(q0, 3)()  # b0 V4-5 + vsums
            fill = deque(units1 + [vterm0, vterm1])
            pv0(0, taker(fill, 2))  # b1 Q + K start
            scores0(1, taker(fill, 1))
            pv0(1, taker(fill, 1))
            scores0(2, taker(fill, 1))  # drains the vterms
            pv0(2, taker(fill, 1))
            scores1(0, None)
            pv1(0, None)
            scores1(1, None)
            pv1(1, None)
            scores1(2, None)
            pv1(2, None, tailmode=True)
        else:
            taker(q0, 19)()
            scores0(0, None)
            scores0(1, None)
            scores0(2, None)
            vterm0()
            pv0(0)
            pv0(1)
            pv0(2)

_CACHE = {}


def _get_program():
    if "nc" not in _CACHE:
        _CACHE["nc"] = _build_program()
    return _CACHE["nc"]


def _make_in_maps(inputs):
    f8 = ml_dtypes.float8_e4m3
    bf = ml_dtypes.bfloat16
    hs = np.asarray(inputs["hidden_states"], np.float32)
    hst = hs.transpose(0, 2, 1)  # [B, H, S]
    x8 = np.ascontiguousarray(
        (hst * SX).reshape(B, KT, P, S).transpose(0, 2, 1, 3).reshape(B, P, KT * S)
    ).astype(f8)
    xt16 = np.ascontiguousarray(
        hst[:, :, NQ:].reshape(B, KT, P, T).transpose(0, 2, 1, 3).reshape(B, P, KT * T)
    ).astype(bf)

    def prep_w(w, scale, dtype, perm=None):  # [out, in] -> W^T tiled
        wt = np.asarray(w, np.float32).T * scale  # [in, out]
        if perm is not None:
            wt = wt[:, perm]
        return np.ascontiguousarray(
            wt.reshape(KT, P, H).transpose(1, 0, 2).reshape(P, KT * H)
        ).astype(dtype)

    bq = np.asarray(inputs["bq"], np.float32) * SC
    bk = np.asarray(inputs["bk"], np.float32) * SC
    bcolqk = np.ascontiguousarray(
        np.concatenate([bq.reshape(KT, P).T, bk.reshape(KT, P).T], axis=1)
    ).astype(np.float32)
    bvb512 = np.tile(
        (np.asarray(inputs["bv"], np.float32) * SC).astype(bf)[None, :], (P, 1)
    )
    in_common = {
        "w8q": prep_w(inputs["Wq"], SWT, f8),
        "w8k": prep_w(inputs["Wk"], SWT, f8),
        "w8v": prep_w(inputs["Wv"], SWT, f8),
        "wv16": prep_w(inputs["Wv"], 1.0, bf),
        "bcolqk": bcolqk,
        "bvb512": bvb512,
    }
    return [
        {
            "x8": x8[i * BL : (i + 1) * BL],
            "xt16": xt16[i * BL : (i + 1) * BL],
            **in_common,
        }
        for i in range(NCORES)
    ]


def kernel(**inputs) -> np.ndarray:
    in_maps = _make_in_maps(inputs)
    nc = _get_program()
    res = run_bass_kernel_spmd(nc, in_maps, list(range(NCORES)))
    return np.concatenate(
        [res.results[i]["out"] for i in range(NCORES)], axis=0
    ).astype(np.float32)


# revision 71
# speedup vs baseline: 1.0295x; 1.0193x over previous
"""Trainium2 Bass kernel for nn_BertSelfAttention_79577154060613.

Block-sparse BERT self-attention (block-diagonal over 10 candidate blocks of
64 tokens + dense global columns for 128 term tokens), data-parallel over
batch across 8 NeuronCores (2 batches per core).

Key algorithmic trick: the reference multiplies scores by the mask (masked
entries become exactly 0, not -inf), so softmax gives each masked key weight
exp(0)=1. For a query in block c:
    ctx = (sum_{k in block c | terms} e^{s_k} v_k + sum_{c' != c} Vsum_c') / Z
    Z   = sum_{k in block c | terms} e^{s_k} + 9*64
where Vsum_c' are per-head, per-block sums of candidate value rows. This
turns 768-wide attention into 192-wide attention plus one small K=10 matmul
(lhsT = 1 - one_hot(c)) per query tile.

The three projections run as fp8e4 DoubleRow matmuls (two fp8 elements per
PE cell -> half the instructions at twice the rate): the host pre-scales x
by 16 and W^T by 32 (power-of-two, exact) and packs the H=768 contraction
as 3 [128 partitions, 2, .] chunk-pair 3D APs. (Scores stay bf16: walrus
rejects DoubleRow combined with PE column tiling, which the 64-partition
block-score matmuls need.) Everything downstream carries the combined 512x
scale: the exp scale folds in 2^-21 exactly, V/Vsums are stored 512x with
the softmax-denominator ones column set to 512 (and 64*512 in Vsums), so
the final reciprocal-multiply cancels the scale with zero extra
instructions. fp8 rounding noise averages
out across the 768-key softmax (<1e-3 relative) everywhere except the term
value rows, which pass through to the output verbatim - those are recomputed
in bf16 (a small [128, H] matmul) with the bias folded in as a rank-1 matmul.

PSUM layout: one [128, 1024] 2-bank tag (psW, 3 bufs) hosts every projection
and score group so each drains with ONE wide ACT/DVE op (engine init
latency ~150-185ns dominates small copies; GpSimd has no PSUM port at all).
A head's scores pack term chunks at [0:512],[512:640] and all 10 block
products at [640:960] - one 960-wide exp per head. PV accumulates in psC
(2 single-bank bufs); each bank hosts its groups opened by full-height
matmuls and closed by a rank-1 +0 whose stop ends the group. The emission
order software-pipelines the two batches: scores/PV stages alternate and
projection units of the other batch fill every gap, so exp work (ACT),
PSUM drains (ACT+DVE split) and matmuls (PE) overlap throughout.
"""

import numpy as np
import ml_dtypes

import concourse.bass as bass
import concourse.mybir as mybir
import concourse.tile as tile
from concourse import bacc
from concourse.bass_utils import run_bass_kernel_spmd

# Problem dims (hardcoded per contract)
B, CDD, L, T, H, NH = 16, 10, 64, 128, 768, 12
DH = H // NH  # 64
S = CDD * L + T  # 768
NQ = CDD * L  # 640
P = 128
NCORES = 8
BL = B // NCORES  # 2 batches per core
KT = H // P  # 6 contraction tiles
KP = KT // 2  # 3 DoubleRow chunk pairs
FP32 = mybir.dt.float32
BF16 = mybir.dt.bfloat16
F8 = mybir.dt.float8e4
AF = mybir.ActivationFunctionType
ALU = mybir.AluOpType
DR = mybir.MatmulPerfMode.DoubleRow
HGS = 4  # heads per attention group (= heads per Q/K o-tile)
NHG = NH // HGS  # 3 groups
VW = DH + 1  # value width per head incl. denominator column (65)
SX = 16.0  # fp8 scale on x
SWT = 32.0  # fp8 scale on W
SC = SX * SWT  # combined 512x scale carried through the kernel
ESCALE = 0.125 / (SC * SC)  # exp scale: 1/sqrt(DH) / 512^2 = 2^-21 (exact)
SB = 640  # block-scores base column inside a head's score tile


def _build_program():
    nc = bacc.Bacc(
        "TRN2", target_bir_lowering=False, debug=False, num_devices=NCORES
    )
    x8 = nc.dram_tensor("x8", [BL, P, KT * S], F8, kind="ExternalInput").ap()
    xt16 = nc.dram_tensor("xt16", [BL, P, KT * T], BF16, kind="ExternalInput").ap()
    w8q = nc.dram_tensor("w8q", [P, KT * H], F8, kind="ExternalInput").ap()
    w8k = nc.dram_tensor("w8k", [P, KT * H], F8, kind="ExternalInput").ap()
    w8v = nc.dram_tensor("w8v", [P, KT * H], F8, kind="ExternalInput").ap()
    wv16 = nc.dram_tensor("wv16", [P, KT * H], BF16, kind="ExternalInput").ap()
    bcolqk = nc.dram_tensor("bcolqk", [P, 2 * KT], FP32, kind="ExternalInput").ap()
    bvb512 = nc.dram_tensor("bvb512", [P, H], BF16, kind="ExternalInput").ap()
    out = nc.dram_tensor("out", [BL, S, H], BF16, kind="ExternalOutput").ap()

    with tile.TileContext(nc) as tc:
        _emit(tc, nc, x8, xt16, w8q, w8k, w8v, wv16, bcolqk, bvb512, out)
    nc.compile()
    return nc


def _emit(tc, nc, x8, xt16, w8q, w8k, w8v, wv16, bcolqk, bvb512, out):
    from collections import deque
    from contextlib import ExitStack

    ctx = ExitStack()
    with ctx:
        cpool = ctx.enter_context(tc.tile_pool(name="consts", bufs=1))
        wpool = ctx.enter_context(tc.tile_pool(name="weights", bufs=1))
        xtp = ctx.enter_context(tc.tile_pool(name="xt", bufs=2))
        qkv = ctx.enter_context(tc.tile_pool(name="qkv", bufs=2))
        sep = ctx.enter_context(tc.tile_pool(name="se", bufs=6))
        osp = ctx.enter_context(tc.tile_pool(name="osb", bufs=2))
        smp = ctx.enter_context(tc.tile_pool(name="small", bufs=2))
        psp = ctx.enter_context(tc.tile_pool(name="psum", bufs=1, space="PSUM"))

        # ---- constants ----
        onesrow = cpool.tile([1, P], BF16)  # 1.0 row (group-closer rank-1 lhsT)
        nc.gpsimd.memset(onesrow[:], 1.0)
        # tiny activation at t=0 pulls the implicit ACT table load into the
        # initial DMA wait instead of blocking the first real copy
        actwarm = cpool.tile([1, P], BF16)
        nc.scalar.activation(actwarm[:], onesrow[:], AF.Exp)
        invrow = cpool.tile([1, P], BF16)  # 1/512 row (vterm bias rank-1 lhsT)
        nc.gpsimd.memset(invrow[:], 1.0 / SC)
        zrow = cpool.tile([1, 1], BF16)  # 0.0 (group-closer rank-1 rhs)
        nc.gpsimd.memset(zrow[:], 0.0)
        # notselC[p, c*64+j] = 0 if p == c else 1  (p in 0..9)
        notselC = cpool.tile([CDD, NQ], BF16)
        nc.gpsimd.memset(notselC[:], 1.0)
        nc.gpsimd.affine_select(
            out=notselC.rearrange("p (c j) -> p c j", j=L),
            in_=notselC.rearrange("p (c j) -> p c j", j=L),
            compare_op=ALU.not_equal,
            fill=0.0,
            base=0,
            pattern=[[-1, CDD], [0, L]],
            channel_multiplier=1,
        )
        # block-membership indicator for Vsums: G[p, j] = 1 iff j-10 == p//64
        G = cpool.tile([P, 20], BF16)
        nc.gpsimd.memset(G[:], 0.0)
        nc.gpsimd.memset(G[0:64, 10:11], 1.0)
        nc.gpsimd.memset(G[64:128, 11:12], 1.0)

        # ---- weights, biases, x (all layouts host-prepared) ----
        wq_sb = wpool.tile([P, KT, H], F8, tag="wq", name="wq")
        wk_sb = wpool.tile([P, KT, H], F8, tag="wk", name="wk")
        wv_sb = wpool.tile([P, KT, H], F8, tag="wv", name="wv")
        wv16_sb = wpool.tile([P, KT, H], BF16, tag="wv16", name="wv16")
        bcol = cpool.tile([P, 2 * KT], FP32)
        bvb = cpool.tile([P, H], BF16)  # 512*bv replicated across partitions
        wqr = w8q.rearrange("p (k o) -> p k o", o=H)
        x8r = [x8[b].rearrange("p (k t) -> p k t", t=S) for b in range(BL)]
        x8t = xtp.tile([P, KT, S], F8, tag="x8", name="x8")
        for c in range(KP):
            cs = slice(2 * c, 2 * c + 2)
            nc.sync.dma_start(out=wq_sb[:, cs, :], in_=wqr[:, cs, :])
            nc.sync.dma_start(out=x8t[:, cs, :], in_=x8r[0][:, cs, :])
        nc.sync.dma_start(out=bcol[:], in_=bcolqk)
        # wk/wv next (batch 0's K/V units need them ~10us in); batch 1's x
        # afterwards (its projections only start mid-run); wv16 last - only
        # vterm needs it
        nc.sync.dma_start(out=wk_sb[:], in_=w8k.rearrange("p (k o) -> p k o", o=H))
        nc.sync.dma_start(out=wv_sb[:], in_=w8v.rearrange("p (k o) -> p k o", o=H))
        nc.sync.dma_start(out=bvb[:], in_=bvb512)
        x8ts = [x8t]
        for b in range(1, BL):
            t8 = xtp.tile([P, KT, S], F8, tag="x8", name="x8")
            nc.sync.dma_start(out=t8[:], in_=x8r[b])
            x8ts.append(t8)
        xt16ts = []
        for b in range(BL):
            t16 = xtp.tile([P, KT, T], BF16, tag="xt16", name="xt16")
            nc.sync.dma_start(
                out=t16[:], in_=xt16[b].rearrange("p (k t) -> p k t", t=T)
            )
            xt16ts.append(t16)
        nc.sync.dma_start(out=wv16_sb[:], in_=wv16.rearrange("p (k o) -> p k o", o=H))

        def psw():
            return psp.tile([P, 1024], FP32, tag="psW", bufs=3, name="psW")

        def emit_proj_units(b):
            """Per-batch projections as a list of small thunks the scheduler
            interleaves with attention stages. Each unit fills a 2-bank psW
            tile ([0:512] bank A, rest bank B) and drains it with ONE wide
            ACT/DVE op. Returns (units, vterm_thunk, tiles)."""
            x8t, xt16t = x8ts[b], xt16ts[b]
            qt_sb = [qkv.tile([P, NQ], BF16, tag=f"qt{m}", name=f"qt{m}") for m in range(KT)]
            kt_sb = [qkv.tile([P, S], BF16, tag=f"kt{m}", name=f"kt{m}") for m in range(KT)]
            vext = [qkv.tile([P, NH * VW], BF16, tag=f"v{m}", name=f"v{m}") for m in range(KT)]
            vterm = qkv.tile([P, H], BF16, tag="vterm", name="vterm")
            vsumsE = smp.tile([CDD, NH * VW], BF16, tag="vsums", name="vsumsE")
            units = []

            def dr_chain(ps, lhs_of, rhs_of, total):
                for n0 in range(0, total, 512):
                    nlen = min(512, total - n0)
                    for c in range(KP):
                        cs = slice(2 * c, 2 * c + 2)
                        nc.tensor.matmul(
                            ps[:, n0 : n0 + nlen],
                            lhsT=lhs_of(cs),
                            rhs=rhs_of(cs, n0, nlen),
                            start=(c == 0),
                            stop=(c == KP - 1),
                            perf_mode=DR,
                        )

            # Q^T, K^T o-tile ot = 2*g4 + i; 512x bias added on the copy
            def qk_unit(wsb, dst, boff, ot, total, on_act):
                ms = slice(ot * P, (ot + 1) * P)
                ps = psw()
                dr_chain(
                    ps,
                    lambda cs: wsb[:, cs, ms],
                    lambda cs, n0, nlen: x8t[:, cs, n0 : n0 + nlen],
                    total,
                )
                bias = bcol[:, boff + ot : boff + ot + 1]
                if on_act:
                    nc.scalar.activation(
                        dst[ot][:], ps[:, 0:total], AF.Identity, bias=bias
                    )
                else:
                    nc.vector.tensor_scalar_add(dst[ot][:], ps[:, 0:total], bias)

            # V: out[t-tile, 0:768] -> vext (bf16 512x, 65-strided)
            def v_unit(mt):
                ms = slice(mt * P, (mt + 1) * P)
                ps = psw()
                dr_chain(
                    ps,
                    lambda cs: x8t[:, cs, ms],
                    lambda cs, n0, nlen: wv_sb[:, cs, n0 : n0 + nlen],
                    H,
                )
                vv = vext[mt].rearrange("p (h c) -> p h c", c=VW)
                nc.vector.tensor_tensor(
                    out=vv[:, :, 0:DH],
                    in0=ps[:, 0:H].rearrange("p (h c) -> p h c", c=DH),
                    in1=bvb.rearrange("p (h c) -> p h c", c=DH),
                    op=ALU.add,
                )
                nc.gpsimd.memset(vv[:, :, DH : DH + 1], SC)

            # per-block value sums (512x), stored with 65th col = 64*512 so
            # the notselC correction matmul also contributes 512*(9*64):
            # heads 0-7 in bank A, 8-11 in bank B, one DVE drain
            def vsum_unit():
                ps = psw()
                for n0, nh0, nh in ((0, 0, 8), (512, 8, 4)):
                    for kt in range(5):
                        rhs = vext[kt].rearrange("p (h c) -> p h c", c=VW)[
                            :, nh0 : nh0 + nh, 0:DH
                        ]
                        nc.tensor.matmul(
                            ps[0:CDD, n0 : n0 + nh * DH],
                            lhsT=G[:, 10 - 2 * kt : 20 - 2 * kt],
                            rhs=rhs,
                            start=(kt == 0),
                            stop=(kt == 4),
                        )
                vsv = vsumsE.rearrange("p (h c) -> p h c", c=VW)
                nc.scalar.activation(
                    vsv[:, :, 0:DH],
                    ps[0:CDD, 0:H].rearrange("p (h c) -> p h c", c=DH),
                    AF.Copy,
                )
                nc.gpsimd.memset(vsv[:, :, DH : DH + 1], float(L) * SC)

            # Q first (needs only wq+x8, which DMA first), then per g4-group
            # K then V so ACT (Q) and DVE (K/V) drain concurrently
            for ot in range(2 * NHG):
                units.append(
                    lambda ot=ot: qk_unit(wq_sb, qt_sb, 0, ot, NQ, ot % 2 == 0)
                )
            for g4 in range(NHG):
                for i in range(2):
                    units.append(
                        lambda ot=2 * g4 + i: qk_unit(
                            wk_sb, kt_sb, KT, ot, S, ot % 2 == 1
                        )
                    )
                units.append(lambda mt=2 * g4: v_unit(mt))
                units.append(lambda mt=2 * g4 + 1: v_unit(mt))
            units.append(vsum_unit)

            # term-value passthrough rows in bf16 (fp8 noise would be exposed
            # verbatim in the output); bias rides a rank-1 (1/512)*bvb matmul
            def vterm_unit():
                ps = psw()
                for n0, nlen in ((0, 512), (512, 256)):
                    for kt in range(KT):
                        nc.tensor.matmul(
                            ps[:, n0 : n0 + nlen],
                            lhsT=xt16t[:, kt, :],
                            rhs=wv16_sb[:, kt, n0 : n0 + nlen],
                            start=(kt == 0),
                            stop=False,
                        )
                    nc.tensor.matmul(
                        ps[:, n0 : n0 + nlen],
                        lhsT=invrow[:],
                        rhs=bvb[0:1, n0 : n0 + nlen],
                        start=False,
                        stop=True,
                    )
                nc.scalar.activation(vterm[:, 0:512], ps[:, 0:512], AF.Copy)
                nc.vector.tensor_copy(vterm[:, 512:H], ps[:, 512:H])
                nc.sync.dma_start(out=out[b][NQ:S, :], in_=vterm[:])

            return units, vterm_unit, (qt_sb, kt_sb, vext, vterm, vsumsE)

        def make_attention(b, tiles):
            """Return (scores, pv) stage emitters for batch b; the top-level
            schedule interleaves them with projection units."""
            qt_sb, kt_sb, vext, vterm, vsumsE = tiles
            obs = [
                osp.tile([P, H], BF16, tag=f"osb{j}", name=f"osb{j}")
                for j in range(5)
            ]
            ses = {}

            def emit_scores(hg, filler=None):
                # per head: one 2-bank tile holds term scores^T at [0:640]
                # and all 10 block products at [640:960]; ONE 960-wide exp.
                # All score matmuls are DoubleRow over (32 partitions, 2).
                se = [sep.tile([P, SB + 5 * L], BF16, tag=f"se{i}", name=f"se{i}") for i in range(HGS)]
                for hl in range(HGS):
                    if filler is not None:
                        filler()
                    hh = hg * HGS + hl
                    pt, r0 = hh // 2, (hh % 2) * 64
                    QTh = qt_sb[pt][r0 : r0 + 64, :]
                    KTh = kt_sb[pt][r0 : r0 + 64, :]
                    ps = psw()
                    for n0, nlen in ((0, 512), (512, 128)):
                        nc.tensor.matmul(
                            ps[:, n0 : n0 + nlen],
                            lhsT=KTh[:, NQ:S],
                            rhs=QTh[:, n0 : n0 + nlen],
                            start=True,
                            stop=True,
                        )
                    for j in range(5):
                        for half in (0, 1):
                            c = 2 * j + half
                            cs = slice(c * L, (c + 1) * L)
                            nc.tensor.matmul(
                                ps[half * 64 : half * 64 + 64, SB + j * L : SB + (j + 1) * L],
                                lhsT=KTh[:, cs],
                                rhs=QTh[:, cs],
                                start=True,
                                stop=True,
                            )
                    nc.scalar.activation(
                        se[hl][:],
                        ps[:, 0 : SB + 5 * L],
                        AF.Exp,
                        scale=ESCALE,
                    )
                ses[hg] = se

            def emit_pv(hg, filler=None, tailmode=False):
                se = ses.pop(hg)
                for j in range(5):
                    if filler is not None:
                        filler()
                    if tailmode and j % 2 == 1:
                        # final stage: psW is quiescent, borrow its banks so
                        # more PV groups are in flight than psC's 2 bufs
                        psc = psw()[:, 0 : HGS * VW]
                    else:
                        psc = psp.tile([P, HGS * VW], FP32, tag="psC", bufs=2, name="psC", padded_shape=[P, 512])
                    hgs_v = slice(hg * HGS * VW, (hg + 1) * HGS * VW)
                    # head 0's full-height terms matmul opens the bank's one
                    # accumulation group; everything else accumulates.
                    for hl in range(HGS):
                        hh = hg * HGS + hl
                        vs = slice(hh * VW, (hh + 1) * VW)
                        nc.tensor.matmul(
                            psc[:, hl * VW : (hl + 1) * VW],
                            lhsT=se[hl][:, j * P : (j + 1) * P],
                            rhs=vext[5][:, vs],
                            start=(hl == 0),
                            stop=False,
                        )
                    # correction: one full-height K=10 matmul per j pair
                    nc.tensor.matmul(
                        psc[:, :],
                        lhsT=notselC[:, (2 * j) * L : (2 * j + 2) * L],
                        rhs=vsumsE[:, hgs_v],
                        start=False,
                        stop=False,
                    )
                    for hl in range(HGS):
                        hh = hg * HGS + hl
                        c0 = hl * VW
                        vs = slice(hh * VW, (hh + 1) * VW)
                        for half in (0, 1):
                            hs = slice(half * 64, half * 64 + 64)
                            nc.tensor.matmul(
                                psc[hs, c0 : c0 + VW],
                                lhsT=se[hl][hs, SB + j * L : SB + (j + 1) * L],
                                rhs=vext[j][hs, vs],
                                start=False,
                                stop=False,
                            )
                    # full-height +0 rank-1 whose stop closes the bank's group
                    nc.tensor.matmul(
                        psc[:, DH : DH + 1],
                        lhsT=onesrow[:],
                        rhs=zrow[:],
                        start=False,
                        stop=True,
                    )
                    zr = smp.tile([P, HGS], FP32, tag="zr", bufs=4, name="zr")
                    pscv = psc.rearrange("p (h c) -> p h c", c=VW)
                    nc.vector.reciprocal(
                        zr[:].rearrange("p (h o) -> p h o", o=1),
                        pscv[:, :, DH : DH + 1],
                    )
                    in0 = pscv[:, :, 0:DH]
                    in1 = zr[:].rearrange("p (h o) -> p h o", o=1)
                    bin0, bin1 = bass.broadcast_tensor_aps(in0, in1)
                    nc.vector.tensor_tensor(
                        out=obs[j][:, hg * HGS * DH : (hg + 1) * HGS * DH].rearrange(
                            "p (h c) -> p h c", c=DH
                        ),
                        in0=bin0,
                        in1=bin1,
                        op=ALU.mult,
                    )
                    if hg == NHG - 1:
                        if b == BL - 1:
                            # tail: ship the first 2 head-groups' columns as
                            # soon as this j's last mult lands; only a third
                            # of the bytes trail the final compute
                            nc.sync.dma_start(
                                out=out[b][j * P : (j + 1) * P, 0 : 2 * HGS * DH],
                                in_=obs[j][:, 0 : 2 * HGS * DH],
                            )
                            nc.sync.dma_start(
                                out=out[b][j * P : (j + 1) * P, 2 * HGS * DH : H],
                                in_=obs[j][:, 2 * HGS * DH : H],
                            )
                        else:
                            nc.sync.dma_start(
                                out=out[b][j * P : (j + 1) * P, :],
                                in_=obs[j][:],
                            )

            return emit_scores, emit_pv

        def taker(q, k):
            def f():
                for _ in range(min(k, len(q))):
                    q.popleft()()
            return f

        # Alternating scores/PV pipeline: each scores stage's 4 exps
        # (ACT-heavy) overlap the neighboring PV stages' matmuls (PE-heavy);
        # batch 1's projections and both vterms fill the gaps in between.
        units0, vterm0, tiles0 = emit_proj_units(0)
        scores0, pv0 = make_attention(0, tiles0)
        q0 = deque(units0)  # 6 Q, 6 K, 6 V, 1 vsums
        if BL > 1:
            units1, vterm1, tiles1 = emit_proj_units(1)
            scores1, pv1 = make_attention(1, tiles1)
            taker(q0, 6)()  # b0 Q (only wq+x8[0] DMAs needed)
            taker(q0, 2)()  # b0 K g4=0
            scores0(0, taker(q0, 2))  # b0 V0-1, K g4=1, V2-3, K g4=2
            taker(q0, 3)()  # b0 V4-5 + vsums
            fill = deque(units1 + [vterm0, vterm1])
            pv0(0, taker(fill, 2))  # b1 Q + K start
            scores0(1, taker(fill, 1))
            pv0(1, taker(fill, 1))
            scores0(2, taker(fill, 1))  # drains the vterms
            pv0(2, taker(fill, 1))
            scores1(0, None)
            pv1(0, None)
            scores1(1, None)
            pv1(1, None)
            scores1(2, None)
            pv1(2, None, tailmode=True)
        else:
            taker(q0, 19)()
            scores0(0, None)
            scores0(1, None)
            scores0(2, None)
            vterm0()
            pv0(0)
            pv0(1)
            pv0(2)

_CACHE = {}


def _get_program():
    if "nc" not in _CACHE:
        _CACHE["nc"] = _build_program()
    return _CACHE["nc"]


def _make_in_maps(inputs):
    f8 = ml_dtypes.float8_e4m3
    bf = ml_dtypes.bfloat16
    hs = np.asarray(inputs["hidden_states"], np.float32)
    hst = hs.transpose(0, 2, 1)  # [B, H, S]
    x8 = np.ascontiguousarray(
        (hst * SX).reshape(B, KT, P, S).transpose(0, 2, 1, 3).reshape(B, P, KT * S)
    ).astype(f8)
    xt16 = np.ascontiguousarray(
        hst[:, :, NQ:].reshape(B, KT, P, T).transpose(0, 2, 1, 3).reshape(B, P, KT * T)
    ).astype(bf)

    def prep_w(w, scale, dtype, perm=None):  # [out, in] -> W^T tiled
        wt = np.asarray(w, np.float32).T * scale  # [in, out]
        if perm is not None:
            wt = wt[:, perm]
        return np.ascontiguousarray(
            wt.reshape(KT, P, H).transpose(1, 0, 2).reshape(P, KT * H)
        ).astype(dtype)

    bq = np.asarray(inputs["bq"], np.float32) * SC
    bk = np.asarray(inputs["bk"], np.float32) * SC
    bcolqk = np.ascontiguousarray(
        np.concatenate([bq.reshape(KT, P).T, bk.reshape(KT, P).T], axis=1)
    ).astype(np.float32)
    bvb512 = np.tile(
        (np.asarray(inputs["bv"], np.float32) * SC).astype(bf)[None, :], (P, 1)
    )
    in_common = {
        "w8q": prep_w(inputs["Wq"], SWT, f8),
        "w8k": prep_w(inputs["Wk"], SWT, f8),
        "w8v": prep_w(inputs["Wv"], SWT, f8),
        "wv16": prep_w(inputs["Wv"], 1.0, bf),
        "bcolqk": bcolqk,
        "bvb512": bvb512,
    }
    return [
        {
            "x8": x8[i * BL : (i + 1) * BL],
            "xt16": xt16[i * BL : (i + 1) * BL],
            **in_common,
        }
        for i in range(NCORES)
    ]


def kernel(**inputs) -> np.ndarray:
    in_maps = _make_in_maps(inputs)
    nc = _get_program()
    res = run_bass_kernel_spmd(nc, in_maps, list(range(NCORES)))
    return np.concatenate(
        [res.results[i]["out"] for i in range(NCORES)], axis=0
    ).astype(np.float32)


# revision 72
# speedup vs baseline: 1.0374x; 1.0077x over previous
"""Trainium2 Bass kernel for nn_BertSelfAttention_79577154060613.

Block-sparse BERT self-attention (block-diagonal over 10 candidate blocks of
64 tokens + dense global columns for 128 term tokens), data-parallel over
batch across 8 NeuronCores (2 batches per core).

Key algorithmic trick: the reference multiplies scores by the mask (masked
entries become exactly 0, not -inf), so softmax gives each masked key weight
exp(0)=1. For a query in block c:
    ctx = (sum_{k in block c | terms} e^{s_k} v_k + sum_{c' != c} Vsum_c') / Z
    Z   = sum_{k in block c | terms} e^{s_k} + 9*64
where Vsum_c' are per-head, per-block sums of candidate value rows. This
turns 768-wide attention into 192-wide attention plus one small K=10 matmul
(lhsT = 1 - one_hot(c)) per query tile.

The three projections run as fp8e4 DoubleRow matmuls (two fp8 elements per
PE cell -> half the instructions at twice the rate): the host pre-scales x
by 16 and W^T by 32 (power-of-two, exact) and packs the H=768 contraction
as 3 [128 partitions, 2, .] chunk-pair 3D APs. (Scores stay bf16: walrus
rejects DoubleRow combined with PE column tiling, which the 64-partition
block-score matmuls need.) Everything downstream carries the combined 512x
scale: the exp scale folds in 2^-21 exactly, V/Vsums are stored 512x with
the softmax-denominator ones column set to 512 (and 64*512 in Vsums), so
the final reciprocal-multiply cancels the scale with zero extra
instructions. fp8 rounding noise averages
out across the 768-key softmax (<1e-3 relative) everywhere except the term
value rows, which pass through to the output verbatim - those are recomputed
in bf16 (a small [128, H] matmul) with the bias folded in as a rank-1 matmul.

PSUM layout: one [128, 1024] 2-bank tag (psW, 3 bufs) hosts every projection
and score group so each drains with ONE wide ACT/DVE op (engine init
latency ~150-185ns dominates small copies; GpSimd has no PSUM port at all).
A head's scores pack term chunks at [0:512],[512:640] and all 10 block
products at [640:960] - one 960-wide exp per head. PV accumulates in psC
(2 single-bank bufs); each bank hosts its groups opened by full-height
matmuls and closed by a rank-1 +0 whose stop ends the group. The emission
order software-pipelines the two batches: scores/PV stages alternate and
projection units of the other batch fill every gap, so exp work (ACT),
PSUM drains (ACT+DVE split) and matmuls (PE) overlap throughout.
"""

import numpy as np
import ml_dtypes

import concourse.bass as bass
import concourse.mybir as mybir
import concourse.tile as tile
from concourse import bacc
from concourse.bass_utils import run_bass_kernel_spmd

# Problem dims (hardcoded per contract)
B, CDD, L, T, H, NH = 16, 10, 64, 128, 768, 12
DH = H // NH  # 64
S = CDD * L + T  # 768
NQ = CDD * L  # 640
P = 128
NCORES = 8
BL = B // NCORES  # 2 batches per core
KT = H // P  # 6 contraction tiles
KP = KT // 2  # 3 DoubleRow chunk pairs
FP32 = mybir.dt.float32
BF16 = mybir.dt.bfloat16
F8 = mybir.dt.float8e4
AF = mybir.ActivationFunctionType
ALU = mybir.AluOpType
DR = mybir.MatmulPerfMode.DoubleRow
HGS = 4  # heads per attention group (= heads per Q/K o-tile)
NHG = NH // HGS  # 3 groups
VW = DH + 1  # value width per head incl. denominator column (65)
SX = 16.0  # fp8 scale on x
SWT = 32.0  # fp8 scale on W
SC = SX * SWT  # combined 512x scale carried through the kernel
ESCALE = 0.125 / (SC * SC)  # exp scale: 1/sqrt(DH) / 512^2 = 2^-21 (exact)
SB = 640  # block-scores base column inside a head's score tile


def _build_program():
    nc = bacc.Bacc(
        "TRN2", target_bir_lowering=False, debug=False, num_devices=NCORES
    )
    x8 = nc.dram_tensor("x8", [BL, P, KT * S], F8, kind="ExternalInput").ap()
    xt16 = nc.dram_tensor("xt16", [BL, P, KT * T], BF16, kind="ExternalInput").ap()
    w8q = nc.dram_tensor("w8q", [P, KT * H], F8, kind="ExternalInput").ap()
    w8k = nc.dram_tensor("w8k", [P, KT * H], F8, kind="ExternalInput").ap()
    w8v = nc.dram_tensor("w8v", [P, KT * H], F8, kind="ExternalInput").ap()
    wv16 = nc.dram_tensor("wv16", [P, KT * H], BF16, kind="ExternalInput").ap()
    bcolqk = nc.dram_tensor("bcolqk", [P, 2 * KT], FP32, kind="ExternalInput").ap()
    bvb512 = nc.dram_tensor("bvb512", [P, H], BF16, kind="ExternalInput").ap()
    out = nc.dram_tensor("out", [BL, S, H], BF16, kind="ExternalOutput").ap()

    with tile.TileContext(nc) as tc:
        _emit(tc, nc, x8, xt16, w8q, w8k, w8v, wv16, bcolqk, bvb512, out)
    nc.compile()
    return nc


def _emit(tc, nc, x8, xt16, w8q, w8k, w8v, wv16, bcolqk, bvb512, out):
    from collections import deque
    from contextlib import ExitStack

    ctx = ExitStack()
    with ctx:
        cpool = ctx.enter_context(tc.tile_pool(name="consts", bufs=1))
        wpool = ctx.enter_context(tc.tile_pool(name="weights", bufs=1))
        xtp = ctx.enter_context(tc.tile_pool(name="xt", bufs=2))
        qkv = ctx.enter_context(tc.tile_pool(name="qkv", bufs=2))
        sep = ctx.enter_context(tc.tile_pool(name="se", bufs=6))
        osp = ctx.enter_context(tc.tile_pool(name="osb", bufs=2))
        smp = ctx.enter_context(tc.tile_pool(name="small", bufs=2))
        psp = ctx.enter_context(tc.tile_pool(name="psum", bufs=1, space="PSUM"))

        # ---- constants ----
        onesrow = cpool.tile([1, P], BF16)  # 1.0 row (group-closer rank-1 lhsT)
        nc.gpsimd.memset(onesrow[:], 1.0)
        # tiny activation at t=0 pulls the implicit ACT table load into the
        # initial DMA wait instead of blocking the first real copy
        actwarm = cpool.tile([1, P], BF16)
        nc.scalar.activation(actwarm[:], onesrow[:], AF.Exp)
        invrow = cpool.tile([1, P], BF16)  # 1/512 row (vterm bias rank-1 lhsT)
        nc.gpsimd.memset(invrow[:], 1.0 / SC)
        zrow = cpool.tile([1, 1], BF16)  # 0.0 (group-closer rank-1 rhs)
        nc.gpsimd.memset(zrow[:], 0.0)
        # notselC[p, c*64+j] = 0 if p == c else 1  (p in 0..9)
        notselC = cpool.tile([CDD, NQ], BF16)
        nc.gpsimd.memset(notselC[:], 1.0)
        nc.gpsimd.affine_select(
            out=notselC.rearrange("p (c j) -> p c j", j=L),
            in_=notselC.rearrange("p (c j) -> p c j", j=L),
            compare_op=ALU.not_equal,
            fill=0.0,
            base=0,
            pattern=[[-1, CDD], [0, L]],
            channel_multiplier=1,
        )
        # block-membership indicator for Vsums: G[p, j] = 1 iff j-10 == p//64
        G = cpool.tile([P, 20], BF16)
        nc.gpsimd.memset(G[:], 0.0)
        nc.gpsimd.memset(G[0:64, 10:11], 1.0)
        nc.gpsimd.memset(G[64:128, 11:12], 1.0)

        # ---- weights, biases, x (all layouts host-prepared) ----
        wq_sb = wpool.tile([P, KT, H], F8, tag="wq", name="wq")
        wk_sb = wpool.tile([P, KT, H], F8, tag="wk", name="wk")
        wv_sb = wpool.tile([P, KT, H], F8, tag="wv", name="wv")
        wv16_sb = wpool.tile([P, KT, H], BF16, tag="wv16", name="wv16")
        bcol = cpool.tile([P, 2 * KT], FP32)
        bvb = cpool.tile([P, H], BF16)  # 512*bv replicated across partitions
        wqr = w8q.rearrange("p (k o) -> p k o", o=H)
        x8r = [x8[b].rearrange("p (k t) -> p k t", t=S) for b in range(BL)]
        x8t = xtp.tile([P, KT, S], F8, tag="x8", name="x8")
        for c in range(KP):
            cs = slice(2 * c, 2 * c + 2)
            nc.sync.dma_start(out=wq_sb[:, cs, :], in_=wqr[:, cs, :])
            nc.sync.dma_start(out=x8t[:, cs, :], in_=x8r[0][:, cs, :])
        nc.sync.dma_start(out=bcol[:], in_=bcolqk)
        # wk/wv next (batch 0's K/V units need them ~10us in); batch 1's x
        # afterwards (its projections only start mid-run); wv16 last - only
        # vterm needs it
        nc.sync.dma_start(out=wk_sb[:], in_=w8k.rearrange("p (k o) -> p k o", o=H))
        nc.sync.dma_start(out=wv_sb[:], in_=w8v.rearrange("p (k o) -> p k o", o=H))
        nc.sync.dma_start(out=bvb[:], in_=bvb512)
        x8ts = [x8t]
        for b in range(1, BL):
            t8 = xtp.tile([P, KT, S], F8, tag="x8", name="x8")
            nc.sync.dma_start(out=t8[:], in_=x8r[b])
            x8ts.append(t8)
        xt16ts = []
        for b in range(BL):
            t16 = xtp.tile([P, KT, T], BF16, tag="xt16", name="xt16")
            nc.sync.dma_start(
                out=t16[:], in_=xt16[b].rearrange("p (k t) -> p k t", t=T)
            )
            xt16ts.append(t16)
        nc.sync.dma_start(out=wv16_sb[:], in_=wv16.rearrange("p (k o) -> p k o", o=H))

        def psw():
            return psp.tile([P, 1024], FP32, tag="psW", bufs=3, name="psW")

        def emit_proj_units(b):
            """Per-batch projections as a list of small thunks the scheduler
            interleaves with attention stages. Each unit fills a 2-bank psW
            tile ([0:512] bank A, rest bank B) and drains it with ONE wide
            ACT/DVE op. Returns (units, vterm_thunk, tiles)."""
            x8t, xt16t = x8ts[b], xt16ts[b]
            qt_sb = [qkv.tile([P, NQ], BF16, tag=f"qt{m}", name=f"qt{m}") for m in range(KT)]
            kt_sb = [qkv.tile([P, S], BF16, tag=f"kt{m}", name=f"kt{m}") for m in range(KT)]
            vext = [qkv.tile([P, NH * VW], BF16, tag=f"v{m}", name=f"v{m}") for m in range(KT)]
            vterm = qkv.tile([P, H], BF16, tag="vterm", name="vterm")
            vsumsE = smp.tile([CDD, NH * VW], BF16, tag="vsums", name="vsumsE")
            units = []

            def dr_chain(ps, lhs_of, rhs_of, total):
                for n0 in range(0, total, 512):
                    nlen = min(512, total - n0)
                    for c in range(KP):
                        cs = slice(2 * c, 2 * c + 2)
                        nc.tensor.matmul(
                            ps[:, n0 : n0 + nlen],
                            lhsT=lhs_of(cs),
                            rhs=rhs_of(cs, n0, nlen),
                            start=(c == 0),
                            stop=(c == KP - 1),
                            perf_mode=DR,
                        )

            # Q^T, K^T o-tile ot = 2*g4 + i; 512x bias added on the copy
            def qk_unit(wsb, dst, boff, ot, total, on_act):
                ms = slice(ot * P, (ot + 1) * P)
                ps = psw()
                dr_chain(
                    ps,
                    lambda cs: wsb[:, cs, ms],
                    lambda cs, n0, nlen: x8t[:, cs, n0 : n0 + nlen],
                    total,
                )
                bias = bcol[:, boff + ot : boff + ot + 1]
                if on_act:
                    nc.scalar.activation(
                        dst[ot][:], ps[:, 0:total], AF.Identity, bias=bias
                    )
                else:
                    nc.vector.tensor_scalar_add(dst[ot][:], ps[:, 0:total], bias)

            # V: out[t-tile, 0:768] -> vext (bf16 512x, 65-strided)
            def v_unit(mt):
                ms = slice(mt * P, (mt + 1) * P)
                ps = psw()
                dr_chain(
                    ps,
                    lambda cs: x8t[:, cs, ms],
                    lambda cs, n0, nlen: wv_sb[:, cs, n0 : n0 + nlen],
                    H,
                )
                vv = vext[mt].rearrange("p (h c) -> p h c", c=VW)
                nc.vector.tensor_tensor(
                    out=vv[:, :, 0:DH],
                    in0=ps[:, 0:H].rearrange("p (h c) -> p h c", c=DH),
                    in1=bvb.rearrange("p (h c) -> p h c", c=DH),
                    op=ALU.add,
                )
                nc.gpsimd.memset(vv[:, :, DH : DH + 1], SC)

            # per-block value sums (512x), stored with 65th col = 64*512 so
            # the notselC correction matmul also contributes 512*(9*64):
            # heads 0-7 in bank A, 8-11 in bank B, one DVE drain
            def vsum_unit():
                ps = psw()
                for n0, nh0, nh in ((0, 0, 8), (512, 8, 4)):
                    for kt in range(5):
                        rhs = vext[kt].rearrange("p (h c) -> p h c", c=VW)[
                            :, nh0 : nh0 + nh, 0:DH
                        ]
                        nc.tensor.matmul(
                            ps[0:CDD, n0 : n0 + nh * DH],
                            lhsT=G[:, 10 - 2 * kt : 20 - 2 * kt],
                            rhs=rhs,
                            start=(kt == 0),
                            stop=(kt == 4),
                        )
                vsv = vsumsE.rearrange("p (h c) -> p h c", c=VW)
                nc.scalar.activation(
                    vsv[:, :, 0:DH],
                    ps[0:CDD, 0:H].rearrange("p (h c) -> p h c", c=DH),
                    AF.Copy,
                )
                nc.gpsimd.memset(vsv[:, :, DH : DH + 1], float(L) * SC)

            # Q first (needs only wq+x8, which DMA first), then per g4-group
            # K then V so ACT (Q) and DVE (K/V) drain concurrently
            for ot in range(2 * NHG):
                units.append(
                    lambda ot=ot: qk_unit(wq_sb, qt_sb, 0, ot, NQ, ot % 3 != 0)
                )
            for g4 in range(NHG):
                for i in range(2):
                    units.append(
                        lambda ot=2 * g4 + i: qk_unit(
                            wk_sb, kt_sb, KT, ot, S, ot % 2 == 0
                        )
                    )
                units.append(lambda mt=2 * g4: v_unit(mt))
                units.append(lambda mt=2 * g4 + 1: v_unit(mt))
            units.append(vsum_unit)

            # term-value passthrough rows in bf16 (fp8 noise would be exposed
            # verbatim in the output); bias rides a rank-1 (1/512)*bvb matmul
            def vterm_unit():
                ps = psw()
                for n0, nlen in ((0, 512), (512, 256)):
                    for kt in range(KT):
                        nc.tensor.matmul(
                            ps[:, n0 : n0 + nlen],
                            lhsT=xt16t[:, kt, :],
                            rhs=wv16_sb[:, kt, n0 : n0 + nlen],
                            start=(kt == 0),
                            stop=False,
                        )
                    nc.tensor.matmul(
                        ps[:, n0 : n0 + nlen],
                        lhsT=invrow[:],
                        rhs=bvb[0:1, n0 : n0 + nlen],
                        start=False,
                        stop=True,
                    )
                nc.scalar.activation(vterm[:, 0:512], ps[:, 0:512], AF.Copy)
                nc.vector.tensor_copy(vterm[:, 512:H], ps[:, 512:H])
                nc.sync.dma_start(out=out[b][NQ:S, :], in_=vterm[:])

            return units, vterm_unit, (qt_sb, kt_sb, vext, vterm, vsumsE)

        def make_attention(b, tiles):
            """Return (scores, pv) stage emitters for batch b; the top-level
            schedule interleaves them with projection units."""
            qt_sb, kt_sb, vext, vterm, vsumsE = tiles
            obs = [
                osp.tile([P, H], BF16, tag=f"osb{j}", name=f"osb{j}")
                for j in range(5)
            ]
            ses = {}

            def emit_scores(hg, filler=None):
                # per head: one 2-bank tile holds term scores^T at [0:640]
                # and all 10 block products at [640:960]; ONE 960-wide exp.
                # All score matmuls are DoubleRow over (32 partitions, 2).
                se = [sep.tile([P, SB + 5 * L], BF16, tag=f"se{i}", name=f"se{i}") for i in range(HGS)]
                for hl in range(HGS):
                    if filler is not None:
                        filler()
                    hh = hg * HGS + hl
                    pt, r0 = hh // 2, (hh % 2) * 64
                    QTh = qt_sb[pt][r0 : r0 + 64, :]
                    KTh = kt_sb[pt][r0 : r0 + 64, :]
                    ps = psw()
                    for n0, nlen in ((0, 512), (512, 128)):
                        nc.tensor.matmul(
                            ps[:, n0 : n0 + nlen],
                            lhsT=KTh[:, NQ:S],
                            rhs=QTh[:, n0 : n0 + nlen],
                            start=True,
                            stop=True,
                        )
                    for j in range(5):
                        for half in (0, 1):
                            c = 2 * j + half
                            cs = slice(c * L, (c + 1) * L)
                            nc.tensor.matmul(
                                ps[half * 64 : half * 64 + 64, SB + j * L : SB + (j + 1) * L],
                                lhsT=KTh[:, cs],
                                rhs=QTh[:, cs],
                                start=True,
                                stop=True,
                            )
                    nc.scalar.activation(
                        se[hl][:],
                        ps[:, 0 : SB + 5 * L],
                        AF.Exp,
                        scale=ESCALE,
                    )
                ses[hg] = se

            def emit_pv(hg, filler=None, tailmode=False):
                se = ses.pop(hg)
                for j in range(5):
                    if filler is not None:
                        filler()
                    if tailmode and j % 2 == 1:
                        # final stage: psW is quiescent, borrow its banks so
                        # more PV groups are in flight than psC's 2 bufs
                        psc = psw()[:, 0 : HGS * VW]
                    else:
                        psc = psp.tile([P, HGS * VW], FP32, tag="psC", bufs=2, name="psC", padded_shape=[P, 512])
                    hgs_v = slice(hg * HGS * VW, (hg + 1) * HGS * VW)
                    # head 0's full-height terms matmul opens the bank's one
                    # accumulation group; everything else accumulates.
                    for hl in range(HGS):
                        hh = hg * HGS + hl
                        vs = slice(hh * VW, (hh + 1) * VW)
                        nc.tensor.matmul(
                            psc[:, hl * VW : (hl + 1) * VW],
                            lhsT=se[hl][:, j * P : (j + 1) * P],
                            rhs=vext[5][:, vs],
                            start=(hl == 0),
                            stop=False,
                        )
                    # correction: one full-height K=10 matmul per j pair
                    nc.tensor.matmul(
                        psc[:, :],
                        lhsT=notselC[:, (2 * j) * L : (2 * j + 2) * L],
                        rhs=vsumsE[:, hgs_v],
                        start=False,
                        stop=False,
                    )
                    for hl in range(HGS):
                        hh = hg * HGS + hl
                        c0 = hl * VW
                        vs = slice(hh * VW, (hh + 1) * VW)
                        for half in (0, 1):
                            hs = slice(half * 64, half * 64 + 64)
                            nc.tensor.matmul(
                                psc[hs, c0 : c0 + VW],
                                lhsT=se[hl][hs, SB + j * L : SB + (j + 1) * L],
                                rhs=vext[j][hs, vs],
                                start=False,
                                stop=False,
                            )
                    # full-height +0 rank-1 whose stop closes the bank's group
                    nc.tensor.matmul(
                        psc[:, DH : DH + 1],
                        lhsT=onesrow[:],
                        rhs=zrow[:],
                        start=False,
                        stop=True,
                    )
                    zr = smp.tile([P, HGS], FP32, tag="zr", bufs=4, name="zr")
                    pscv = psc.rearrange("p (h c) -> p h c", c=VW)
                    nc.vector.reciprocal(
                        zr[:].rearrange("p (h o) -> p h o", o=1),
                        pscv[:, :, DH : DH + 1],
                    )
                    in0 = pscv[:, :, 0:DH]
                    in1 = zr[:].rearrange("p (h o) -> p h o", o=1)
                    bin0, bin1 = bass.broadcast_tensor_aps(in0, in1)
                    nc.vector.tensor_tensor(
                        out=obs[j][:, hg * HGS * DH : (hg + 1) * HGS * DH].rearrange(
                            "p (h c) -> p h c", c=DH
                        ),
                        in0=bin0,
                        in1=bin1,
                        op=ALU.mult,
                    )
                    if hg == NHG - 1:
                        if b == BL - 1:
                            # tail: ship the first 2 head-groups' columns as
                            # soon as this j's last mult lands; only a third
                            # of the bytes trail the final compute
                            nc.sync.dma_start(
                                out=out[b][j * P : (j + 1) * P, 0 : 2 * HGS * DH],
                                in_=obs[j][:, 0 : 2 * HGS * DH],
                            )
                            nc.sync.dma_start(
                                out=out[b][j * P : (j + 1) * P, 2 * HGS * DH : H],
                                in_=obs[j][:, 2 * HGS * DH : H],
                            )
                        else:
                            nc.sync.dma_start(
                                out=out[b][j * P : (j + 1) * P, :],
                                in_=obs[j][:],
                            )

            return emit_scores, emit_pv

        def taker(q, k):
            def f():
                for _ in range(min(k, len(q))):
                    q.popleft()()
            return f

        # Alternating scores/PV pipeline: each scores stage's 4 exps
        # (ACT-heavy) overlap the neighboring PV stages' matmuls (PE-heavy);
        # batch 1's projections and both vterms fill the gaps in between.
        units0, vterm0, tiles0 = emit_proj_units(0)
        scores0, pv0 = make_attention(0, tiles0)
        q0 = deque(units0)  # 6 Q, 6 K, 6 V, 1 vsums
        if BL > 1:
            units1, vterm1, tiles1 = emit_proj_units(1)
            scores1, pv1 = make_attention(1, tiles1)
            taker(q0, 6)()  # b0 Q (only wq+x8[0] DMAs needed)
            taker(q0, 2)()  # b0 K g4=0
            scores0(0, taker(q0, 2))  # b0 V0-1, K g4=1, V2-3, K g4=2
            taker(q0, 3)()  # b0 V4-5 + vsums
            fill = deque(units1 + [vterm0, vterm1])
            pv0(0, taker(fill, 2))  # b1 Q + K start
            scores0(1, taker(fill, 1))
            pv0(1, taker(fill, 1))
            scores0(2, taker(fill, 1))  # drains the vterms
            pv0(2, taker(fill, 1))
            scores1(0, None)
            pv1(0, None)
            scores1(1, None)
            pv1(1, None)
            scores1(2, None)
            pv1(2, None, tailmode=True)
        else:
            taker(q0, 19)()
            scores0(0, None)
            scores0(1, None)
            scores0(2, None)
            vterm0()
            pv0(0)
            pv0(1)
            pv0(2)

_CACHE = {}


def _get_program():
    if "nc" not in _CACHE:
        _CACHE["nc"] = _build_program()
    return _CACHE["nc"]


def _make_in_maps(inputs):
    f8 = ml_dtypes.float8_e4m3
    bf = ml_dtypes.bfloat16
    hs = np.asarray(inputs["hidden_states"], np.float32)
    hst = hs.transpose(0, 2, 1)  # [B, H, S]
    x8 = np.ascontiguousarray(
        (hst * SX).reshape(B, KT, P, S).transpose(0, 2, 1, 3).reshape(B, P, KT * S)
    ).astype(f8)
    xt16 = np.ascontiguousarray(
        hst[:, :, NQ:].reshape(B, KT, P, T).transpose(0, 2, 1, 3).reshape(B, P, KT * T)
    ).astype(bf)

    def prep_w(w, scale, dtype, perm=None):  # [out, in] -> W^T tiled
        wt = np.asarray(w, np.float32).T * scale  # [in, out]
        if perm is not None:
            wt = wt[:, perm]
        return np.ascontiguousarray(
            wt.reshape(KT, P, H).transpose(1, 0, 2).reshape(P, KT * H)
        ).astype(dtype)

    bq = np.asarray(inputs["bq"], np.float32) * SC
    bk = np.asarray(inputs["bk"], np.float32) * SC
    bcolqk = np.ascontiguousarray(
        np.concatenate([bq.reshape(KT, P).T, bk.reshape(KT, P).T], axis=1)
    ).astype(np.float32)
    bvb512 = np.tile(
        (np.asarray(inputs["bv"], np.float32) * SC).astype(bf)[None, :], (P, 1)
    )
    in_common = {
        "w8q": prep_w(inputs["Wq"], SWT, f8),
        "w8k": prep_w(inputs["Wk"], SWT, f8),
        "w8v": prep_w(inputs["Wv"], SWT, f8),
        "wv16": prep_w(inputs["Wv"], 1.0, bf),
        "bcolqk": bcolqk,
        "bvb512": bvb512,
    }
    return [
        {
            "x8": x8[i * BL : (i + 1) * BL],
            "xt16": xt16[i * BL : (i + 1) * BL],
            **in_common,
        }
        for i in range(NCORES)
    ]


def kernel(**inputs) -> np.ndarray:
    in_maps = _make_in_maps(inputs)
    nc = _get_program()
    res = run_bass_kernel_spmd(nc, in_maps, list(range(NCORES)))
    return np.concatenate(
        [res.results[i]["out"] for i in range(NCORES)], axis=0
    ).astype(np.float32)


# revision 74
# speedup vs baseline: 1.0516x; 1.0137x over previous
"""Trainium2 Bass kernel for nn_BertSelfAttention_79577154060613.

Block-sparse BERT self-attention (block-diagonal over 10 candidate blocks of
64 tokens + dense global columns for 128 term tokens), data-parallel over
batch across 8 NeuronCores (2 batches per core).

Key algorithmic trick: the reference multiplies scores by the mask (masked
entries become exactly 0, not -inf), so softmax gives each masked key weight
exp(0)=1. For a query in block c:
    ctx = (sum_{k in block c | terms} e^{s_k} v_k + sum_{c' != c} Vsum_c') / Z
    Z   = sum_{k in block c | terms} e^{s_k} + 9*64
where Vsum_c' are per-head, per-block sums of candidate value rows. This
turns 768-wide attention into 192-wide attention plus one small K=10 matmul
(lhsT = 1 - one_hot(c)) per query tile.

The three projections run as fp8e4 DoubleRow matmuls (two fp8 elements per
PE cell -> half the instructions at twice the rate): the host pre-scales x
by 16 and W^T by 32 (power-of-two, exact) and packs the H=768 contraction
as 3 [128 partitions, 2, .] chunk-pair 3D APs. (Scores stay bf16: walrus
rejects DoubleRow combined with PE column tiling, which the 64-partition
block-score matmuls need.) Everything downstream carries the combined 512x
scale: the exp scale folds in 2^-21 exactly, V/Vsums are stored 512x with
the softmax-denominator ones column set to 512 (and 64*512 in Vsums), so
the final reciprocal-multiply cancels the scale with zero extra
instructions. fp8 rounding noise averages
out across the 768-key softmax (<1e-3 relative) everywhere except the term
value rows, which pass through to the output verbatim - those are recomputed
in bf16 (a small [128, H] matmul) with the bias folded in as a rank-1 matmul.

PSUM layout: one [128, 1024] 2-bank tag (psW, 3 bufs) hosts every projection
and score group so each drains with ONE wide ACT/DVE op (engine init
latency ~150-185ns dominates small copies; GpSimd has no PSUM port at all).
A head's scores pack term chunks at [0:512],[512:640] and all 10 block
products at [640:960] - one 960-wide exp per head. PV accumulates in psC
(2 single-bank bufs); each bank hosts its groups opened by full-height
matmuls and closed by a rank-1 +0 whose stop ends the group. The emission
order software-pipelines the two batches: scores/PV stages alternate and
projection units of the other batch fill every gap, so exp work (ACT),
PSUM drains (ACT+DVE split) and matmuls (PE) overlap throughout.
"""

import numpy as np
import ml_dtypes

import concourse.bass as bass
import concourse.mybir as mybir
import concourse.tile as tile
from concourse import bacc
from concourse.bass_utils import run_bass_kernel_spmd

# Problem dims (hardcoded per contract)
B, CDD, L, T, H, NH = 16, 10, 64, 128, 768, 12
DH = H // NH  # 64
S = CDD * L + T  # 768
NQ = CDD * L  # 640
P = 128
NCORES = 8
BL = B // NCORES  # 2 batches per core
KT = H // P  # 6 contraction tiles
KP = KT // 2  # 3 DoubleRow chunk pairs
FP32 = mybir.dt.float32
BF16 = mybir.dt.bfloat16
F8 = mybir.dt.float8e4
AF = mybir.ActivationFunctionType
ALU = mybir.AluOpType
DR = mybir.MatmulPerfMode.DoubleRow
HGS = 4  # heads per attention group (= heads per Q/K o-tile)
NHG = NH // HGS  # 3 groups
VW = DH + 1  # value width per head incl. denominator column (65)
SX = 16.0  # fp8 scale on x
SWT = 32.0  # fp8 scale on W
SC = SX * SWT  # combined 512x scale carried through the kernel
ESCALE = 0.125 / (SC * SC)  # exp scale: 1/sqrt(DH) / 512^2 = 2^-21 (exact)
SB = 640  # block-scores base column inside a head's score tile


def _build_program():
    nc = bacc.Bacc(
        "TRN2", target_bir_lowering=False, debug=False, num_devices=NCORES
    )
    x8 = nc.dram_tensor("x8", [BL, P, KT * S], F8, kind="ExternalInput").ap()
    xlo8 = nc.dram_tensor("xlo8", [BL, P, KT * T], F8, kind="ExternalInput").ap()
    w8q = nc.dram_tensor("w8q", [P, KT * H], F8, kind="ExternalInput").ap()
    w8k = nc.dram_tensor("w8k", [P, KT * H], F8, kind="ExternalInput").ap()
    w8v = nc.dram_tensor("w8v", [P, KT * H], F8, kind="ExternalInput").ap()
    wlo8 = nc.dram_tensor("wlo8", [P, KT * H], F8, kind="ExternalInput").ap()
    bcolqk = nc.dram_tensor("bcolqk", [P, 2 * KT], FP32, kind="ExternalInput").ap()
    bvb512 = nc.dram_tensor("bvb512", [P, H], BF16, kind="ExternalInput").ap()
    out = nc.dram_tensor("out", [BL, S, H], BF16, kind="ExternalOutput").ap()

    with tile.TileContext(nc) as tc:
        _emit(tc, nc, x8, xlo8, w8q, w8k, w8v, wlo8, bcolqk, bvb512, out)
    nc.compile()
    return nc


def _emit(tc, nc, x8, xlo8, w8q, w8k, w8v, wlo8, bcolqk, bvb512, out):
    from collections import deque
    from contextlib import ExitStack

    ctx = ExitStack()
    with ctx:
        cpool = ctx.enter_context(tc.tile_pool(name="consts", bufs=1))
        wpool = ctx.enter_context(tc.tile_pool(name="weights", bufs=1))
        xtp = ctx.enter_context(tc.tile_pool(name="xt", bufs=2))
        qkv = ctx.enter_context(tc.tile_pool(name="qkv", bufs=2))
        sep = ctx.enter_context(tc.tile_pool(name="se", bufs=6))
        osp = ctx.enter_context(tc.tile_pool(name="osb", bufs=2))
        smp = ctx.enter_context(tc.tile_pool(name="small", bufs=2))
        psp = ctx.enter_context(tc.tile_pool(name="psum", bufs=1, space="PSUM"))

        # ---- constants ----
        onesrow = cpool.tile([1, P], BF16)  # 1.0 row (group-closer rank-1 lhsT)
        nc.gpsimd.memset(onesrow[:], 1.0)
        # tiny activation at t=0 pulls the implicit ACT table load into the
        # initial DMA wait instead of blocking the first real copy
        actwarm = cpool.tile([1, P], BF16)
        nc.scalar.activation(actwarm[:], onesrow[:], AF.Exp)
        invrow = cpool.tile([1, P], BF16)  # 1/512 row (vterm bias rank-1 lhsT)
        nc.gpsimd.memset(invrow[:], 1.0 / SC)
        zrow = cpool.tile([1, 1], BF16)  # 0.0 (group-closer rank-1 rhs)
        nc.gpsimd.memset(zrow[:], 0.0)
        # notselC[p, c*64+j] = 0 if p == c else 1  (p in 0..9)
        notselC = cpool.tile([CDD, NQ], BF16)
        nc.gpsimd.memset(notselC[:], 1.0)
        nc.gpsimd.affine_select(
            out=notselC.rearrange("p (c j) -> p c j", j=L),
            in_=notselC.rearrange("p (c j) -> p c j", j=L),
            compare_op=ALU.not_equal,
            fill=0.0,
            base=0,
            pattern=[[-1, CDD], [0, L]],
            channel_multiplier=1,
        )
        # block-membership indicator for Vsums: G[p, j] = 1 iff j-10 == p//64
        G = cpool.tile([P, 20], BF16)
        nc.gpsimd.memset(G[:], 0.0)
        nc.gpsimd.memset(G[0:64, 10:11], 1.0)
        nc.gpsimd.memset(G[64:128, 11:12], 1.0)

        # ---- weights, biases, x (all layouts host-prepared) ----
        wq_sb = wpool.tile([P, KT, H], F8, tag="wq", name="wq")
        wk_sb = wpool.tile([P, KT, H], F8, tag="wk", name="wk")
        wv_sb = wpool.tile([P, KT, H], F8, tag="wv", name="wv")
        wlo_sb = wpool.tile([P, KT, H], F8, tag="wlo", name="wlo")
        bcol = cpool.tile([P, 2 * KT], FP32)
        bvb = cpool.tile([P, H], BF16)  # 512*bv replicated across partitions
        wqr = w8q.rearrange("p (k o) -> p k o", o=H)
        x8r = [x8[b].rearrange("p (k t) -> p k t", t=S) for b in range(BL)]
        x8t = xtp.tile([P, KT, S], F8, tag="x8", name="x8")
        for c in range(KP):
            cs = slice(2 * c, 2 * c + 2)
            nc.sync.dma_start(out=wq_sb[:, cs, :], in_=wqr[:, cs, :])
            nc.sync.dma_start(out=x8t[:, cs, :], in_=x8r[0][:, cs, :])
        nc.sync.dma_start(out=bcol[:], in_=bcolqk)
        # wk/wv next (batch 0's K/V units need them ~10us in); batch 1's x
        # afterwards (its projections only start mid-run); wv16 last - only
        # vterm needs it
        nc.sync.dma_start(out=wk_sb[:], in_=w8k.rearrange("p (k o) -> p k o", o=H))
        nc.sync.dma_start(out=wv_sb[:], in_=w8v.rearrange("p (k o) -> p k o", o=H))
        nc.sync.dma_start(out=bvb[:], in_=bvb512)
        x8ts = [x8t]
        for b in range(1, BL):
            t8 = xtp.tile([P, KT, S], F8, tag="x8", name="x8")
            nc.sync.dma_start(out=t8[:], in_=x8r[b])
            x8ts.append(t8)
        xlo8ts = []
        for b in range(BL):
            tlo = xtp.tile([P, KT, T], F8, tag="xlo8", name="xlo8")
            nc.sync.dma_start(
                out=tlo[:], in_=xlo8[b].rearrange("p (k t) -> p k t", t=T)
            )
            xlo8ts.append(tlo)
        nc.sync.dma_start(out=wlo_sb[:], in_=wlo8.rearrange("p (k o) -> p k o", o=H))

        def psw():
            return psp.tile([P, 1024], FP32, tag="psW", bufs=3, name="psW")

        def emit_proj_units(b):
            """Per-batch projections as a list of small thunks the scheduler
            interleaves with attention stages. Each unit fills a 2-bank psW
            tile ([0:512] bank A, rest bank B) and drains it with ONE wide
            ACT/DVE op. Returns (units, vterm_thunk, tiles)."""
            x8t, xlo8t = x8ts[b], xlo8ts[b]
            qt_sb = [qkv.tile([P, NQ], BF16, tag=f"qt{m}", name=f"qt{m}") for m in range(KT)]
            kt_sb = [qkv.tile([P, S], BF16, tag=f"kt{m}", name=f"kt{m}") for m in range(KT)]
            vext = [qkv.tile([P, NH * VW], BF16, tag=f"v{m}", name=f"v{m}") for m in range(KT)]
            vterm = qkv.tile([P, H], BF16, tag="vterm", name="vterm")
            vsumsE = smp.tile([CDD, NH * VW], BF16, tag="vsums", name="vsumsE")
            units = []

            def dr_chain(ps, lhs_of, rhs_of, total):
                for n0 in range(0, total, 512):
                    nlen = min(512, total - n0)
                    for c in range(KP):
                        cs = slice(2 * c, 2 * c + 2)
                        nc.tensor.matmul(
                            ps[:, n0 : n0 + nlen],
                            lhsT=lhs_of(cs),
                            rhs=rhs_of(cs, n0, nlen),
                            start=(c == 0),
                            stop=(c == KP - 1),
                            perf_mode=DR,
                        )

            # Q^T, K^T o-tile ot = 2*g4 + i; 512x bias added on the copy
            def qk_unit(wsb, dst, boff, ot, total, on_act):
                ms = slice(ot * P, (ot + 1) * P)
                ps = psw()
                dr_chain(
                    ps,
                    lambda cs: wsb[:, cs, ms],
                    lambda cs, n0, nlen: x8t[:, cs, n0 : n0 + nlen],
                    total,
                )
                bias = bcol[:, boff + ot : boff + ot + 1]
                if on_act:
                    nc.scalar.activation(
                        dst[ot][:], ps[:, 0:total], AF.Identity, bias=bias
                    )
                else:
                    nc.vector.tensor_scalar_add(dst[ot][:], ps[:, 0:total], bias)

            # V: out[t-tile, 0:768] -> vext (bf16 512x, 65-strided)
            def v_unit(mt):
                ms = slice(mt * P, (mt + 1) * P)
                ps = psw()
                dr_chain(
                    ps,
                    lambda cs: x8t[:, cs, ms],
                    lambda cs, n0, nlen: wv_sb[:, cs, n0 : n0 + nlen],
                    H,
                )
                vv = vext[mt].rearrange("p (h c) -> p h c", c=VW)
                nc.vector.tensor_tensor(
                    out=vv[:, :, 0:DH],
                    in0=ps[:, 0:H].rearrange("p (h c) -> p h c", c=DH),
                    in1=bvb.rearrange("p (h c) -> p h c", c=DH),
                    op=ALU.add,
                )
                nc.gpsimd.memset(vv[:, :, DH : DH + 1], SC)

            # per-block value sums (512x), stored with 65th col = 64*512 so
            # the notselC correction matmul also contributes 512*(9*64):
            # heads 0-7 in bank A, 8-11 in bank B, one DVE drain
            def vsum_unit():
                ps = psw()
                for n0, nh0, nh in ((0, 0, 8), (512, 8, 4)):
                    for kt in range(5):
                        rhs = vext[kt].rearrange("p (h c) -> p h c", c=VW)[
                            :, nh0 : nh0 + nh, 0:DH
                        ]
                        nc.tensor.matmul(
                            ps[0:CDD, n0 : n0 + nh * DH],
                            lhsT=G[:, 10 - 2 * kt : 20 - 2 * kt],
                            rhs=rhs,
                            start=(kt == 0),
                            stop=(kt == 4),
                        )
                vsv = vsumsE.rearrange("p (h c) -> p h c", c=VW)
                nc.scalar.activation(
                    vsv[:, :, 0:DH],
                    ps[0:CDD, 0:H].rearrange("p (h c) -> p h c", c=DH),
                    AF.Copy,
                )
                nc.gpsimd.memset(vsv[:, :, DH : DH + 1], float(L) * SC)

            # Q first (needs only wq+x8, which DMA first), then per g4-group
            # K then V so ACT (Q) and DVE (K/V) drain concurrently
            for ot in range(2 * NHG):
                units.append(
                    lambda ot=ot: qk_unit(wq_sb, qt_sb, 0, ot, NQ, ot % 3 != 0)
                )
            for g4 in range(NHG):
                for i in range(2):
                    units.append(
                        lambda ot=2 * g4 + i: qk_unit(
                            wk_sb, kt_sb, KT, ot, S, ot % 2 == 0
                        )
                    )
                units.append(lambda mt=2 * g4: v_unit(mt))
                units.append(lambda mt=2 * g4 + 1: v_unit(mt))
            units.append(vsum_unit)

            # term-value passthrough rows: single fp8 would expose ~3%
            # quantization verbatim in the output, so split-precision fp8:
            # x16 ~ x8 + xlo, w32 ~ w8v + wlo (residuals quantized at scale
            # 1), and psum = x8*w8v + x8*wlo + xlo*w8v - three DoubleRow
            # chains at the common 512x scale (dropped lolo term ~2^-8);
            # bias rides the rank-1 (1/512)*bvb matmul
            def vterm_unit():
                ps = psw()
                xterm = x8t[:, :, NQ:S]
                for n0, nlen in ((0, 512), (512, 256)):
                    first = True
                    for lhs, rhsw in (
                        (xterm, wv_sb),
                        (xterm, wlo_sb),
                        (xlo8t, wv_sb),
                    ):
                        for c in range(KP):
                            cs = slice(2 * c, 2 * c + 2)
                            nc.tensor.matmul(
                                ps[:, n0 : n0 + nlen],
                                lhsT=lhs[:, cs, :],
                                rhs=rhsw[:, cs, n0 : n0 + nlen],
                                start=first,
                                stop=False,
                                perf_mode=DR,
                            )
                            first = False
                    nc.tensor.matmul(
                        ps[:, n0 : n0 + nlen],
                        lhsT=onesrow[:],
                        rhs=bvb[0:1, n0 : n0 + nlen],
                        start=False,
                        stop=True,
                    )
                nc.scalar.activation(
                    vterm[:, 0:512], ps[:, 0:512], AF.Copy, scale=1.0 / SC
                )
                nc.vector.tensor_scalar_mul(
                    vterm[:, 512:H], ps[:, 512:H], 1.0 / SC
                )
                nc.sync.dma_start(out=out[b][NQ:S, :], in_=vterm[:])

            return units, vterm_unit, (qt_sb, kt_sb, vext, vterm, vsumsE)

        def make_attention(b, tiles):
            """Return (scores, pv) stage emitters for batch b; the top-level
            schedule interleaves them with projection units."""
            qt_sb, kt_sb, vext, vterm, vsumsE = tiles
            obs = [
                osp.tile([P, H], BF16, tag=f"osb{j}", name=f"osb{j}")
                for j in range(5)
            ]
            ses = {}

            def emit_scores(hg, filler=None):
                # per head: one 2-bank tile holds term scores^T at [0:640]
                # and all 10 block products at [640:960]; ONE 960-wide exp.
                # All score matmuls are DoubleRow over (32 partitions, 2).
                se = [sep.tile([P, SB + 5 * L], BF16, tag=f"se{i}", name=f"se{i}") for i in range(HGS)]
                for hl in range(HGS):
                    if filler is not None:
                        filler()
                    hh = hg * HGS + hl
                    pt, r0 = hh // 2, (hh % 2) * 64
                    QTh = qt_sb[pt][r0 : r0 + 64, :]
                    KTh = kt_sb[pt][r0 : r0 + 64, :]
                    ps = psw()
                    for n0, nlen in ((0, 512), (512, 128)):
                        nc.tensor.matmul(
                            ps[:, n0 : n0 + nlen],
                            lhsT=KTh[:, NQ:S],
                            rhs=QTh[:, n0 : n0 + nlen],
                            start=True,
                            stop=True,
                        )
                    for j in range(5):
                        for half in (0, 1):
                            c = 2 * j + half
                            cs = slice(c * L, (c + 1) * L)
                            nc.tensor.matmul(
                                ps[half * 64 : half * 64 + 64, SB + j * L : SB + (j + 1) * L],
                                lhsT=KTh[:, cs],
                                rhs=QTh[:, cs],
                                start=True,
                                stop=True,
                            )
                    nc.scalar.activation(
                        se[hl][:],
                        ps[:, 0 : SB + 5 * L],
                        AF.Exp,
                        scale=ESCALE,
                    )
                ses[hg] = se

            def emit_pv(hg, filler=None, tailmode=False):
                se = ses.pop(hg)
                for j in range(5):
                    if filler is not None:
                        filler()
                    if tailmode and j % 2 == 1:
                        # final stage: psW is quiescent, borrow its banks so
                        # more PV groups are in flight than psC's 2 bufs
                        psc = psw()[:, 0 : HGS * VW]
                    else:
                        psc = psp.tile([P, HGS * VW], FP32, tag="psC", bufs=2, name="psC", padded_shape=[P, 512])
                    hgs_v = slice(hg * HGS * VW, (hg + 1) * HGS * VW)
                    # head 0's full-height terms matmul opens the bank's one
                    # accumulation group; everything else accumulates.
                    for hl in range(HGS):
                        hh = hg * HGS + hl
                        vs = slice(hh * VW, (hh + 1) * VW)
                        nc.tensor.matmul(
                            psc[:, hl * VW : (hl + 1) * VW],
                            lhsT=se[hl][:, j * P : (j + 1) * P],
                            rhs=vext[5][:, vs],
                            start=(hl == 0),
                            stop=False,
                        )
                    # correction: one full-height K=10 matmul per j pair
                    nc.tensor.matmul(
                        psc[:, :],
                        lhsT=notselC[:, (2 * j) * L : (2 * j + 2) * L],
                        rhs=vsumsE[:, hgs_v],
                        start=False,
                        stop=False,
                    )
                    for hl in range(HGS):
                        hh = hg * HGS + hl
                        c0 = hl * VW
                        vs = slice(hh * VW, (hh + 1) * VW)
                        for half in (0, 1):
                            hs = slice(half * 64, half * 64 + 64)
                            nc.tensor.matmul(
                                psc[hs, c0 : c0 + VW],
                                lhsT=se[hl][hs, SB + j * L : SB + (j + 1) * L],
                                rhs=vext[j][hs, vs],
                                start=False,
                                stop=False,
                            )
                    # full-height +0 rank-1 whose stop closes the bank's group
                    nc.tensor.matmul(
                        psc[:, DH : DH + 1],
                        lhsT=onesrow[:],
                        rhs=zrow[:],
                        start=False,
                        stop=True,
                    )
                    zr = smp.tile([P, HGS], FP32, tag="zr", bufs=4, name="zr")
                    pscv = psc.rearrange("p (h c) -> p h c", c=VW)
                    nc.vector.reciprocal(
                        zr[:].rearrange("p (h o) -> p h o", o=1),
                        pscv[:, :, DH : DH + 1],
                    )
                    in0 = pscv[:, :, 0:DH]
                    in1 = zr[:].rearrange("p (h o) -> p h o", o=1)
                    bin0, bin1 = bass.broadcast_tensor_aps(in0, in1)
                    nc.vector.tensor_tensor(
                        out=obs[j][:, hg * HGS * DH : (hg + 1) * HGS * DH].rearrange(
                            "p (h c) -> p h c", c=DH
                        ),
                        in0=bin0,
                        in1=bin1,
                        op=ALU.mult,
                    )
                    if hg == NHG - 1:
                        if b == BL - 1:
                            # tail: ship the first 2 head-groups' columns as
                            # soon as this j's last mult lands; only a third
                            # of the bytes trail the final compute
                            nc.sync.dma_start(
                                out=out[b][j * P : (j + 1) * P, 0 : 2 * HGS * DH],
                                in_=obs[j][:, 0 : 2 * HGS * DH],
                            )
                            nc.sync.dma_start(
                                out=out[b][j * P : (j + 1) * P, 2 * HGS * DH : H],
                                in_=obs[j][:, 2 * HGS * DH : H],
                            )
                        else:
                            nc.sync.dma_start(
                                out=out[b][j * P : (j + 1) * P, :],
                                in_=obs[j][:],
                            )

            return emit_scores, emit_pv

        def taker(q, k):
            def f():
                for _ in range(min(k, len(q))):
                    q.popleft()()
            return f

        # Alternating scores/PV pipeline: each scores stage's 4 exps
        # (ACT-heavy) overlap the neighboring PV stages' matmuls (PE-heavy);
        # batch 1's projections and both vterms fill the gaps in between.
        units0, vterm0, tiles0 = emit_proj_units(0)
        scores0, pv0 = make_attention(0, tiles0)
        q0 = deque(units0)  # 6 Q, 6 K, 6 V, 1 vsums
        if BL > 1:
            units1, vterm1, tiles1 = emit_proj_units(1)
            scores1, pv1 = make_attention(1, tiles1)
            taker(q0, 6)()  # b0 Q (only wq+x8[0] DMAs needed)
            taker(q0, 2)()  # b0 K g4=0
            scores0(0, taker(q0, 2))  # b0 V0-1, K g4=1, V2-3, K g4=2
            taker(q0, 3)()  # b0 V4-5 + vsums
            fill = deque(units1 + [vterm0, vterm1])
            pv0(0, taker(fill, 2))  # b1 Q + K start
            scores0(1, taker(fill, 1))
            pv0(1, taker(fill, 1))
            scores0(2, taker(fill, 1))  # drains the vterms
            pv0(2, taker(fill, 1))
            scores1(0, None)
            pv1(0, None)
            scores1(1, None)
            pv1(1, None)
            scores1(2, None)
            pv1(2, None, tailmode=True)
        else:
            taker(q0, 19)()
            scores0(0, None)
            scores0(1, None)
            scores0(2, None)
            vterm0()
            pv0(0)
            pv0(1)
            pv0(2)

_CACHE = {}


def _get_program():
    if "nc" not in _CACHE:
        _CACHE["nc"] = _build_program()
    return _CACHE["nc"]


def _make_in_maps(inputs):
    f8 = ml_dtypes.float8_e4m3
    bf = ml_dtypes.bfloat16
    hs = np.asarray(inputs["hidden_states"], np.float32)
    hst = hs.transpose(0, 2, 1)  # [B, H, S]
    x8 = np.ascontiguousarray(
        (hst * SX).reshape(B, KT, P, S).transpose(0, 2, 1, 3).reshape(B, P, KT * S)
    ).astype(f8)
    x16term = np.ascontiguousarray(
        (hst[:, :, NQ:] * SX)
        .reshape(B, KT, P, T)
        .transpose(0, 2, 1, 3)
        .reshape(B, P, KT * T)
    )
    xlo8 = (x16term - x16term.astype(f8).astype(np.float32)).astype(f8)

    def prep_w(w, scale, dtype, perm=None):  # [out, in] -> W^T tiled
        wt = np.asarray(w, np.float32).T * scale  # [in, out]
        if perm is not None:
            wt = wt[:, perm]
        return np.ascontiguousarray(
            wt.reshape(KT, P, H).transpose(1, 0, 2).reshape(P, KT * H)
        ).astype(dtype)

    bq = np.asarray(inputs["bq"], np.float32) * SC
    bk = np.asarray(inputs["bk"], np.float32) * SC
    bcolqk = np.ascontiguousarray(
        np.concatenate([bq.reshape(KT, P).T, bk.reshape(KT, P).T], axis=1)
    ).astype(np.float32)
    bvb512 = np.tile(
        (np.asarray(inputs["bv"], np.float32) * SC).astype(bf)[None, :], (P, 1)
    )
    in_common = {
        "w8q": prep_w(inputs["Wq"], SWT, f8),
        "w8k": prep_w(inputs["Wk"], SWT, f8),
        "w8v": prep_w(inputs["Wv"], SWT, f8),
        "wlo8": (
            lambda w32: (w32 - w32.astype(f8).astype(np.float32)).astype(f8)
        )(prep_w(inputs["Wv"], SWT, np.float32)),
        "bcolqk": bcolqk,
        "bvb512": bvb512,
    }
    return [
        {
            "x8": x8[i * BL : (i + 1) * BL],
            "xlo8": xlo8[i * BL : (i + 1) * BL],
            **in_common,
        }
        for i in range(NCORES)
    ]


def kernel(**inputs) -> np.ndarray:
    in_maps = _make_in_maps(inputs)
    nc = _get_program()
    res = run_bass_kernel_spmd(nc, in_maps, list(range(NCORES)))
    return np.concatenate(
        [res.results[i]["out"] for i in range(NCORES)], axis=0
    ).astype(np.float32)
